# revision 1
# baseline (speedup 1.0000x reference)
"""DecoderAttention Bass/Tile kernel for TRN2, batch-parallel over 8 NeuronCores.

Each core handles one batch element:
  q = enc @ Qs + Qbs ; k = enc @ Ks + Kbs ; v = nrp @ Vs + Vbs   (per head)
  scores = q k^T / sqrt(64), causal mask (-1e5), softmax
  out = (attn @ v) @ O + Ob

Layout strategy (matmuls in fp32r at full PE rate):
  - enc/nrp transposed on-device (PE transpose) to [d, s]
  - weights pre-packed host-side to [d, (h dh)]; Vs padded to [d, 16*65]
    with a ones column per head so attn@v also produces softmax row sums
  - scoresT [m, q] per head so exp output feeds attn@v without transposing
  - causal diagonal blocks masked by a GpSimd affine_select zeroing exp output
  - exp folds the 1/sqrt(d_head) scale; no max subtraction (scores are O(1),
    masked entries become exactly 0)
  - q/k projections for pair g+1 are interleaved into pair g's attention as
    PE filler work, so the tensor engine never idles long enough for the HAM
    clock gate to re-throttle it to 1.2 GHz
  - softmax division deferred: one batched reciprocal at the end, broadcast
    across partitions with one-hot selector matmuls
"""

import numpy as np

import concourse.bass as bass
import concourse.mybir as mybir
import concourse.tile as tile
from concourse import bacc
from concourse.bass_utils import run_bass_kernel_spmd
from concourse.masks import make_identity

N_HEADS, D_MODEL, D_HEAD = 16, 1024, 64
BATCH, SEQ = 8, 1024
P = 128
DCH = D_MODEL // P       # 8 contraction chunks
ST = SEQ // P            # 8 seq tiles
PAIRS = N_HEADS // 2     # 8 head pairs
VW = 65                  # v width per head incl. ones column
VTOT = N_HEADS * VW      # 1040
IGNORE = -100000.0
SCALE = 1.0 / np.sqrt(np.float32(D_HEAD))

F32 = mybir.dt.float32
F32R = mybir.dt.float32r
BF16 = mybir.dt.bfloat16
AF = mybir.ActivationFunctionType

_CACHE = {}


def _bank_splits(q0):
    # PSUM-bank-aligned (n0, nw) column splits covering [q0, SEQ)
    if q0 < 512:
        return [(q0, 512 - q0), (512, 512)]
    return [(q0, SEQ - q0)]


def _bcast_row_ap(src, n):
    # DMA access pattern replicating a [n]-element DRAM row to 128 partitions
    return bass.AP(tensor=src.tensor, offset=src.offset, ap=[[0, P], [1, n]])


def _build_program(debug=False):
    nc = bacc.Bacc("TRN2", target_bir_lowering=False, debug=False, num_devices=8)

    enc = nc.dram_tensor("enc", [SEQ, D_MODEL], F32R, kind="ExternalInput").ap()
    nrp = nc.dram_tensor("nrp", [SEQ, D_MODEL], F32R, kind="ExternalInput").ap()
    qst = nc.dram_tensor("qst", [D_MODEL, D_MODEL], F32R, kind="ExternalInput").ap()
    kst = nc.dram_tensor("kst", [D_MODEL, D_MODEL], F32R, kind="ExternalInput").ap()
    vst = nc.dram_tensor("vst", [D_MODEL, VTOT], F32R, kind="ExternalInput").ap()
    ow = nc.dram_tensor("ow", [D_MODEL, D_MODEL], F32R, kind="ExternalInput").ap()
    qb = nc.dram_tensor("qb", [D_MODEL], F32, kind="ExternalInput").ap()
    kb = nc.dram_tensor("kb", [D_MODEL], F32, kind="ExternalInput").ap()
    vb = nc.dram_tensor("vb", [VTOT], F32, kind="ExternalInput").ap()
    ob = nc.dram_tensor("ob", [D_MODEL], F32, kind="ExternalInput").ap()
    out = nc.dram_tensor("out", [SEQ, D_MODEL], F32, kind="ExternalOutput").ap()
    sums_dram = nc.dram_tensor("sums_scratch", [N_HEADS, SEQ], F32).ap()
    rcp_dram = nc.dram_tensor("rcp_scratch", [P, P], F32R).ap()
    dbg = None
    if debug:
        dbg = {
            "qt0": nc.dram_tensor("d_qt0", [P, SEQ], F32, kind="ExternalOutput").ap(),
            "kt0": nc.dram_tensor("d_kt0", [P, SEQ], F32, kind="ExternalOutput").ap(),
            "va0": nc.dram_tensor("d_va0", [P, VTOT], F32, kind="ExternalOutput").ap(),
            "zt": nc.dram_tensor("d_zt", [DCH, P, SEQ], F32, kind="ExternalOutput").ap(),
        }

    with tile.TileContext(nc) as tc:
        _kernel(tc, out, enc, nrp, qst, kst, vst, ow, qb, kb, vb, ob,
                sums_dram=sums_dram, rcp_dram=rcp_dram, dbg=dbg)
    nc.compile()
    return nc


def _kernel(tc, out, enc, nrp, qst, kst, vst, ow, qb, kb, vb, ob,
            sums_dram=None, rcp_dram=None, dbg=None):
    nc = tc.nc

    smalls = tc.alloc_tile_pool(name="smalls", bufs=1)
    identf = smalls.tile([P, P], F32, tag="identf", name="identf")
    make_identity(nc, identf)
    ident = smalls.tile([P, P], F32R, tag="ident", name="ident")
    nc.vector.tensor_copy(ident, identf)
    ident_bf = smalls.tile([P, P], BF16, tag="ident_bf", name="ident_bf")
    make_identity(nc, ident_bf)
    # M0[m, q] = IGNORE where m > q else 0 (strict causal mask, diag block)
    mask_bf = smalls.tile([P, P], BF16, tag="mask_bf", name="mask_bf")
    nc.gpsimd.memset(mask_bf, 0.0)
    nc.gpsimd.affine_select(
        out=mask_bf, in_=mask_bf,
        compare_op=mybir.AluOpType.is_ge,
        fill=IGNORE, base=0,
        pattern=[[1, P]], channel_multiplier=-1,
    )
    vb_bc = smalls.tile([P, VTOT], F32, tag="vb_bc", name="vb_bc")
    nc.sync.dma_start(out=vb_bc, in_=_bcast_row_ap(vb, VTOT))
    ob_bc = smalls.tile([P, D_MODEL], F32, tag="ob_bc", name="ob_bc")
    nc.sync.dma_start(out=ob_bc, in_=_bcast_row_ap(ob, D_MODEL))
    qb_col = smalls.tile([P, PAIRS], F32, tag="qb_col", name="qb_col")
    nc.sync.dma_start(out=qb_col, in_=qb.rearrange("(g p) -> p g", p=P))
    kb_col = smalls.tile([P, PAIRS], F32, tag="kb_col", name="kb_col")
    nc.sync.dma_start(out=kb_col, in_=kb.rearrange("(g p) -> p g", p=P))

    enc_t_pool = tc.alloc_tile_pool(name="encT", bufs=1, side="right")
    nrp_t_pool = tc.alloc_tile_pool(name="nrpT", bufs=1, side="right")
    encT = [enc_t_pool.tile([P, SEQ], F32R, tag=f"encT{c}", name=f"encT{c}") for c in range(DCH)]
    nrpT = [nrp_t_pool.tile([P, SEQ], F32R, tag=f"nrpT{c}", name=f"nrpT{c}") for c in range(DCH)]

    # ---- phase 1: transpose enc and nrp into [d, s] ----
    with tc.tile_pool(name="trin", bufs=2) as trin, \
         tc.tile_pool(name="trps", bufs=1, space="PSUM") as trps:
        for src, dst in ((enc, encT), (nrp, nrpT)):
            for tq in range(0, ST, 4):
                ptiles = [trps.tile([P, 4 * P], F32R, tag=f"tr{c}", name=f"tr{c}") for c in range(DCH)]
                for t in range(tq, tq + 4):
                    s_in = trin.tile([P, D_MODEL], F32R, tag="s_in", name="s_in")
                    nc.sync.dma_start(out=s_in, in_=src[t * P:(t + 1) * P, :])
                    for c in range(DCH):
                        nc.tensor.transpose(
                            ptiles[c][:, (t - tq) * P:(t - tq + 1) * P],
                            s_in[:, c * P:(c + 1) * P],
                            ident,
                        )
                for c in range(DCH):
                    nc.any.tensor_copy(dst[c][:, tq * P:(tq + 4) * P], ptiles[c])

    # ---- phase 2ab: q/k projections (dedicated phase, weights prefetched) ----
    qt_pool = tc.alloc_tile_pool(name="qt", bufs=1)
    kt_pool = tc.alloc_tile_pool(name="kt", bufs=1)
    qt = [qt_pool.tile([P, SEQ], F32R, tag=f"qt{g}", name=f"qt{g}") for g in range(PAIRS)]
    kt = [kt_pool.tile([P, SEQ], F32R, tag=f"kt{g}", name=f"kt{g}") for g in range(PAIRS)]
    with tc.tile_pool(name="wsb", bufs=1) as wsb, \
         tc.tile_pool(name="pproj", bufs=1, space="PSUM") as pproj:
        wqk = {}
        for pfx, wsrc in (("q", qst), ("k", kst)):
            w = [wsb.tile([P, D_MODEL], F32R, tag=f"{pfx}w{c}", name=f"{pfx}w{c}") for c in range(DCH)]
            for c in range(DCH):
                nc.scalar.dma_start(out=w[c], in_=wsrc[c * P:(c + 1) * P, :])
            wqk[pfx] = w
        for pfx, bcol, dsts in (("q", qb_col, qt), ("k", kb_col, kt)):
            w = wqk[pfx]
            for n0 in range(0, SEQ, 512):
                ptiles = [pproj.tile([P, 512], F32, tag=f"pp{g}", name=f"pp{g}") for g in range(PAIRS)]
                for c in range(DCH):
                    for g in range(PAIRS):
                        nc.tensor.matmul(
                            ptiles[g],
                            w[c][:, g * P:(g + 1) * P],
                            encT[c][:, n0:n0 + 512],
                            start=(c == 0), stop=(c == DCH - 1),
                        )
                for g in range(PAIRS):
                    nc.vector.tensor_scalar_add(
                        out=dsts[g][:, n0:n0 + 512],
                        in0=ptiles[g],
                        scalar1=bcol[:, g:g + 1],
                    )

    # ---- phase 2: v projection -> va [m, 16*65] with ones columns ----
    va_pool = tc.alloc_tile_pool(name="va", bufs=1)
    va = [va_pool.tile([P, VTOT], F32R, tag=f"va{t}", name=f"va{t}") for t in range(ST)]
    with tc.tile_pool(name="vsb", bufs=1) as vsb, \
         tc.tile_pool(name="pv", bufs=2, space="PSUM") as pv:
        vw = [vsb.tile([P, VTOT], F32R, tag=f"vw{c}", name=f"vw{c}") for c in range(DCH)]
        for c in range(DCH):
            nc.scalar.dma_start(out=vw[c], in_=vst[c * P:(c + 1) * P, :])
        for t in range(ST):
            pt = pv.tile([P, VTOT], F32, tag="pv", name="pvt")
            for c in range(DCH):
                for n0 in range(0, VTOT, 512):
                    nw = min(512, VTOT - n0)
                    nc.tensor.matmul(
                        pt[:, n0:n0 + nw],
                        nrpT[c][:, t * P:(t + 1) * P],
                        vw[c][:, n0:n0 + nw],
                        start=(c == 0), stop=(c == DCH - 1),
                    )
            # vb_bc has the per-(h,dh) bias, with 1.0 in each ones-column slot;
            # matmul wrote 0 there (vst ones-columns are zero), so add gives 1.0
            nc.vector.tensor_add(va[t], pt, vb_bc)

    nrp_t_pool.release()
    enc_t_pool.release()

    # ---- phase 3: attention, with next pair's q/k projection interleaved ----
    zt_pool = tc.alloc_tile_pool(name="zt", bufs=1)
    zt = [zt_pool.tile([P, SEQ], F32R, tag=f"zt{k}", name=f"zt{k}") for k in range(DCH)]
    osb = tc.alloc_tile_pool(name="osb", bufs=1)
    owt = [osb.tile([P, D_MODEL], F32R, tag=f"ow{k}", name=f"owt{k}") for k in range(DCH)]
    for k in range(DCH):
        nc.scalar.dma_start(out=owt[k], in_=ow[k * P:(k + 1) * P, :])

    with tc.tile_pool(name="attn", bufs=3) as apool, \
         tc.tile_pool(name="rcp", bufs=1) as rpool, \
         tc.tile_pool(name="selp", bufs=1) as selp, \
         tc.tile_pool(name="ps_s", bufs=2, space="PSUM") as spool, \
         tc.tile_pool(name="ps_z", bufs=2, space="PSUM") as zpool:
        # sel[g][j, p] = 1 where j == 2g + p // 64  (K=16 one-hot broadcast)
        sel = []
        for b in range(PAIRS):
            self_f = selp.tile([N_HEADS, P], F32, tag="self", name="self", bufs=2)
            nc.gpsimd.memset(self_f, 0.0)
            nc.gpsimd.affine_select(
                out=self_f.rearrange("j (a c) -> j a c", a=2),
                in_=self_f.rearrange("j (a c) -> j a c", a=2),
                compare_op=mybir.AluOpType.not_equal,
                fill=1.0, base=-2 * b,
                pattern=[[-1, 2], [0, D_HEAD]], channel_multiplier=1,
            )
            s_r = selp.tile([N_HEADS, P], F32R, tag=f"sel{b}", name=f"sel{b}")
            nc.vector.tensor_copy(s_r, self_f)
            sel.append(s_r)

        for h in range(N_HEADS):
            g, off = h // 2, (h % 2) * D_HEAD
            pz = zpool.tile([VW, SEQ], F32, tag="pz", name="pz")

            def av_mms(i, ae):
                q0 = i * P
                for n0, nw in _bank_splits(q0):
                    nc.tensor.matmul(
                        pz[:, n0:n0 + nw],
                        va[i][:, h * VW:(h + 1) * VW],
                        ae[:, n0:n0 + nw],
                        start=(i == 0), stop=(i == ST - 1),
                        skip_group_check=True,
                    )

            pend = None
            for i in range(ST):
                q0 = i * P
                ps = spool.tile([P, SEQ], F32, tag="ps", name="ps")
                ae = apool.tile([P, SEQ], F32R, tag="ae", name="ae")
                for n0, nw in _bank_splits(q0):
                    nc.tensor.matmul(
                        ps[:, n0:n0 + nw],
                        kt[g][off:off + D_HEAD, q0:q0 + P],
                        qt[g][off:off + D_HEAD, n0:n0 + nw],
                        start=True, stop=(n0 != q0),
                        skip_group_check=True,
                    )
                # causal diag mask: accumulate I.T @ M0
                nc.tensor.matmul(
                    ps[:, q0:q0 + P],
                    ident_bf, mask_bf,
                    start=False, stop=True,
                    skip_group_check=True,
                )
                nc.scalar.activation(
                    out=ae[:, q0:SEQ], in_=ps[:, q0:SEQ],
                    func=AF.Exp, scale=float(SCALE),
                )
                # attn@v delayed one chunk so exp latency hides behind PE work
                if pend is not None:
                    av_mms(*pend)
                pend = (i, ae)
            av_mms(*pend)
            # stash unnormalized zT and the denominator row; frees PSUM slots
            nc.vector.tensor_copy(zt[g][off:off + D_HEAD, :], pz[0:D_HEAD, :])
            srow = rpool.tile([1, SEQ], F32, tag="srow", name="srow", bufs=2)
            nc.scalar.copy(out=srow, in_=pz[D_HEAD:VW, :])
            nc.sync.dma_start(out=sums_dram[h:h + 1, :], in_=srow)

        # normalize: reciprocal over the sums reshaped to [128, 128] so the
        # FD-bound iterative divide runs across partitions (1.3us vs 6.5us),
        # then reload in [16, SEQ] layout for the broadcast matmuls
        s128 = rpool.tile([P, P], F32, tag="s128", name="s128")
        nc.sync.dma_start(out=s128, in_=sums_dram.rearrange("h (a c) -> (h a) c", c=P))
        r128 = rpool.tile([P, P], F32R, tag="r128", name="r128")
        with nc.allow_low_precision(reason="softmax denominators are O(1); fp32r rounding is fine"):
            nc.vector.reciprocal(out=r128, in_=s128)
        nc.sync.dma_start(out=rcp_dram, in_=r128)
        r16 = rpool.tile([N_HEADS, SEQ], F32R, tag="r16", name="r16")
        nc.sync.dma_start(out=r16, in_=rcp_dram.rearrange("(h a) c -> h (a c)", h=N_HEADS))
        for gg in range(PAIRS):
            pb = spool.tile([P, SEQ], F32, tag="ps", name="psb")
            for n0 in (0, 512):
                nc.tensor.matmul(
                    pb[:, n0:n0 + 512], sel[gg], r16[:, n0:n0 + 512],
                    start=True, stop=True,
                )
            nc.vector.tensor_mul(zt[gg], zt[gg], pb)

    if dbg is not None:
        nc.sync.dma_start(out=dbg["va0"], in_=va[0].bitcast(F32))
        for k in range(DCH):
            nc.sync.dma_start(out=dbg["zt"][k], in_=zt[k].bitcast(F32))

    # ---- phase 4: output projection out[s, d] = zt.T @ O + ob ----
    with tc.tile_pool(name="outsb", bufs=3) as outsb, \
         tc.tile_pool(name="po", bufs=2, space="PSUM") as po:
        for t in range(ST):
            pt = po.tile([P, D_MODEL], F32, tag="po", name="pot")
            for k in range(DCH):
                for n0 in range(0, D_MODEL, 512):
                    nc.tensor.matmul(
                        pt[:, n0:n0 + 512],
                        zt[k][:, t * P:(t + 1) * P],
                        owt[k][:, n0:n0 + 512],
                        start=(k == 0), stop=(k == DCH - 1),
                    )
            ot = outsb.tile([P, D_MODEL], F32, tag="ot", name="ot")
            nc.vector.tensor_add(ot, pt, ob_bc)
            nc.sync.dma_start(out=out[t * P:(t + 1) * P, :], in_=ot)

    for pool in (osb, zt_pool, va_pool, kt_pool, qt_pool, smalls):
        pool.release()


def _get_program():
    if "nc" not in _CACHE:
        _CACHE["nc"] = _build_program()
    return _CACHE["nc"]


def _pack_weights(Qs, Qbs, Ks, Kbs, Vs, Vbs, O, Ob):
    f = np.float32
    qst = np.ascontiguousarray(np.transpose(np.asarray(Qs, f), (1, 0, 2)).reshape(D_MODEL, D_MODEL))
    kst = np.ascontiguousarray(np.transpose(np.asarray(Ks, f), (1, 0, 2)).reshape(D_MODEL, D_MODEL))
    vst = np.zeros((D_MODEL, VTOT), f)
    vb = np.zeros((VTOT,), f)
    Vs = np.asarray(Vs, f)
    Vbs = np.asarray(Vbs, f)
    for h in range(N_HEADS):
        vst[:, h * VW:h * VW + D_HEAD] = Vs[h]
        vb[h * VW:h * VW + D_HEAD] = Vbs[h]
        vb[h * VW + D_HEAD] = 1.0
    ow = np.ascontiguousarray(np.asarray(O, f).reshape(D_MODEL, D_MODEL))
    qbf = np.ascontiguousarray(np.asarray(Qbs, f).reshape(D_MODEL))
    kbf = np.ascontiguousarray(np.asarray(Kbs, f).reshape(D_MODEL))
    obf = np.ascontiguousarray(np.asarray(Ob, f).reshape(D_MODEL))
    return qst, kst, vst, ow, qbf, kbf, vb, obf


def kernel(normalized_resid_pre, encoder_output, Qs, Qbs, Ks, Kbs, Vs, Vbs, O, Ob,
           _trace=False, _trace_kwargs=None):
    nc = _get_program()
    qst, kst, vst, ow, qbf, kbf, vb, obf = _pack_weights(Qs, Qbs, Ks, Kbs, Vs, Vbs, O, Ob)
    enc = np.asarray(encoder_output, np.float32)
    nrp = np.asarray(normalized_resid_pre, np.float32)
    in_maps = []
    for b in range(BATCH):
        in_maps.append({
            "enc": np.ascontiguousarray(enc[b]),
            "nrp": np.ascontiguousarray(nrp[b]),
            "qst": qst, "kst": kst, "vst": vst, "ow": ow,
            "qb": qbf, "kb": kbf, "vb": vb, "ob": obf,
        })
    res = run_bass_kernel_spmd(
        nc, in_maps, list(range(BATCH)),
        trace=_trace, **(_trace_kwargs or {}),
    )
    out = np.stack([res.results[b]["out"] for b in range(BATCH)], axis=0)
    if _trace:
        _CACHE["last_results"] = res
    return out



# revision 3
# speedup vs baseline: 1.2929x; 1.2929x over previous
"""DecoderAttention Bass/Tile kernel for TRN2, batch-parallel over 8 NeuronCores.

Each core handles one batch element:
  q = enc @ Qs + Qbs ; k = enc @ Ks + Kbs ; v = nrp @ Vs + Vbs   (per head)
  scores = q k^T / sqrt(64), causal mask (-1e5), softmax
  out = (attn @ v) @ O + Ob

Layout/throughput strategy (all matmuls in bf16 at 1 col/cycle, f32 PSUM):
  - all big DRAM inputs pre-cast to bf16 host-side (halves DMA, enables
    bf16 PE transposes and 2x matmul rate vs fp32r; measured end-to-end
    max rel err ~4e-3 vs the 2e-2 gate)
  - enc/nrp transposed on-device (PE transpose, bf16 PSUM) to [d, s]
  - weights pre-packed host-side to [d, (h dh)]; Vs padded to [d, 16*65]
    with a ones column per head so attn@v also produces softmax row sums
  - scoresT [m, q] per head so exp output feeds attn@v without transposing
  - causal diagonal blocks masked by accumulating I.T @ M0 (bf16) in PSUM
  - exp folds the 1/sqrt(d_head) scale; no max subtraction (scores are O(1),
    masked entries become exactly 0)
  - HAM: the PE clock gate needs gapless activity to hold 2.4 GHz, so the
    q/k projection for pair g+1 is interleaved into pair g's attention as
    PE filler (2 matmuls per m-tile), and pair g-1's softmax normalization
    (reciprocal broadcast matmul + multiply) rides the same filler queue
  - softmax division deferred: per-pair reciprocal over sums reshaped to
    [16, 128] (partition-parallel), broadcast across partitions with a
    K=2 one-hot selector matmul, applied to zt while later pairs run
"""

import numpy as np
import ml_dtypes

import concourse.bass as bass
import concourse.mybir as mybir
import concourse.tile as tile
from concourse import bacc
from concourse.bass_utils import run_bass_kernel_spmd
from concourse.masks import make_identity

N_HEADS, D_MODEL, D_HEAD = 16, 1024, 64
BATCH, SEQ = 8, 1024
P = 128
DCH = D_MODEL // P       # 8 contraction chunks
ST = SEQ // P            # 8 seq tiles
PAIRS = N_HEADS // 2     # 8 head pairs
VW = 65                  # v width per head incl. ones column
VTOT = N_HEADS * VW      # 1040
IGNORE = -100000.0
SCALE = 1.0 / np.sqrt(np.float32(D_HEAD))

F32 = mybir.dt.float32
F32R = mybir.dt.float32r
BF16 = mybir.dt.bfloat16
AF = mybir.ActivationFunctionType
NPBF16 = ml_dtypes.bfloat16

_CACHE = {}


def _bank_splits(q0):
    # PSUM-bank-aligned (n0, nw) column splits covering [q0, SEQ)
    if q0 < 512:
        return [(q0, 512 - q0), (512, 512)]
    return [(q0, SEQ - q0)]


def _bcast_row_ap(src, n):
    # DMA access pattern replicating a [n]-element DRAM row to 128 partitions
    return bass.AP(tensor=src.tensor, offset=src.offset, ap=[[0, P], [1, n]])


def _build_program():
    nc = bacc.Bacc("TRN2", target_bir_lowering=False, debug=False, num_devices=8)

    enc = nc.dram_tensor("enc", [SEQ, D_MODEL], BF16, kind="ExternalInput").ap()
    nrp = nc.dram_tensor("nrp", [SEQ, D_MODEL], BF16, kind="ExternalInput").ap()
    qst = nc.dram_tensor("qst", [D_MODEL, D_MODEL], BF16, kind="ExternalInput").ap()
    kst = nc.dram_tensor("kst", [D_MODEL, D_MODEL], BF16, kind="ExternalInput").ap()
    vst = nc.dram_tensor("vst", [D_MODEL, VTOT], BF16, kind="ExternalInput").ap()
    ow = nc.dram_tensor("ow", [D_MODEL, D_MODEL], BF16, kind="ExternalInput").ap()
    qb = nc.dram_tensor("qb", [D_MODEL], F32, kind="ExternalInput").ap()
    kb = nc.dram_tensor("kb", [D_MODEL], F32, kind="ExternalInput").ap()
    vb = nc.dram_tensor("vb", [VTOT], F32, kind="ExternalInput").ap()
    ob = nc.dram_tensor("ob", [D_MODEL], F32, kind="ExternalInput").ap()
    out = nc.dram_tensor("out", [SEQ, D_MODEL], F32, kind="ExternalOutput").ap()
    sums_dram = nc.dram_tensor("sums_scratch", [N_HEADS, SEQ], F32).ap()
    rcp_dram = nc.dram_tensor("rcp_scratch", [PAIRS, N_HEADS, P], F32R).ap()

    with tile.TileContext(nc) as tc:
        _kernel(tc, out, enc, nrp, qst, kst, vst, ow, qb, kb, vb, ob,
                sums_dram=sums_dram, rcp_dram=rcp_dram)
    nc.compile()
    return nc


def _kernel(tc, out, enc, nrp, qst, kst, vst, ow, qb, kb, vb, ob,
            sums_dram=None, rcp_dram=None):
    nc = tc.nc

    smalls = tc.alloc_tile_pool(name="smalls", bufs=1)
    ident_bf = smalls.tile([P, P], BF16, tag="ident_bf", name="ident_bf")
    make_identity(nc, ident_bf)
    # M0[m, q] = IGNORE where m > q else 0 (strict causal mask, diag block)
    mask_bf = smalls.tile([P, P], BF16, tag="mask_bf", name="mask_bf")
    nc.gpsimd.memset(mask_bf, 0.0)
    nc.gpsimd.affine_select(
        out=mask_bf, in_=mask_bf,
        compare_op=mybir.AluOpType.is_ge,
        fill=IGNORE, base=0,
        pattern=[[1, P]], channel_multiplier=-1,
    )
    # sel2[j, p] = 1 where j == p // 64 (K=2 one-hot broadcast for rcp rows)
    sel2f = smalls.tile([2, P], F32, tag="sel2f", name="sel2f")
    nc.gpsimd.memset(sel2f, 0.0)
    nc.gpsimd.affine_select(
        out=sel2f.rearrange("j (a c) -> j a c", a=2),
        in_=sel2f.rearrange("j (a c) -> j a c", a=2),
        compare_op=mybir.AluOpType.not_equal,
        fill=1.0, base=0,
        pattern=[[-1, 2], [0, D_HEAD]], channel_multiplier=1,
    )
    sel2 = smalls.tile([2, P], F32R, tag="sel2", name="sel2")
    nc.vector.tensor_copy(sel2, sel2f)
    vb_bc = smalls.tile([P, VTOT], F32, tag="vb_bc", name="vb_bc")
    nc.sync.dma_start(out=vb_bc, in_=_bcast_row_ap(vb, VTOT))
    ob_bc = smalls.tile([P, D_MODEL], F32, tag="ob_bc", name="ob_bc")
    nc.sync.dma_start(out=ob_bc, in_=_bcast_row_ap(ob, D_MODEL))
    qb_col = smalls.tile([P, PAIRS], F32, tag="qb_col", name="qb_col")
    nc.sync.dma_start(out=qb_col, in_=qb.rearrange("(g p) -> p g", p=P))
    kb_col = smalls.tile([P, PAIRS], F32, tag="kb_col", name="kb_col")
    nc.sync.dma_start(out=kb_col, in_=kb.rearrange("(g p) -> p g", p=P))

    # persistent weight tiles (bf16), prefetched on the gpsimd DMA queue;
    # vst first since the v projection consumes it earliest
    wpool = tc.alloc_tile_pool(name="weights", bufs=1, side="right")
    vw = [wpool.tile([P, VTOT], BF16, tag=f"vw{c}", name=f"vw{c}") for c in range(DCH)]
    qw = [wpool.tile([P, D_MODEL], BF16, tag=f"qw{c}", name=f"qw{c}") for c in range(DCH)]
    kw = [wpool.tile([P, D_MODEL], BF16, tag=f"kw{c}", name=f"kw{c}") for c in range(DCH)]
    owt = [wpool.tile([P, D_MODEL], BF16, tag=f"owt{c}", name=f"owt{c}") for c in range(DCH)]
    for c in range(DCH):
        nc.gpsimd.dma_start(out=vw[c], in_=vst[c * P:(c + 1) * P, :])
    for c in range(DCH):
        nc.gpsimd.dma_start(out=qw[c], in_=qst[c * P:(c + 1) * P, :])
        nc.gpsimd.dma_start(out=kw[c], in_=kst[c * P:(c + 1) * P, :])
    for c in range(DCH):
        nc.gpsimd.dma_start(out=owt[c], in_=ow[c * P:(c + 1) * P, :])

    enc_t_pool = tc.alloc_tile_pool(name="encT", bufs=1, side="right")
    nrp_t_pool = tc.alloc_tile_pool(name="nrpT", bufs=1, side="right")
    encT = [enc_t_pool.tile([P, SEQ], BF16, tag=f"encT{c}", name=f"encT{c}") for c in range(DCH)]
    nrpT = [nrp_t_pool.tile([P, SEQ], BF16, tag=f"nrpT{c}", name=f"nrpT{c}") for c in range(DCH)]

    # ---- phase 1: transpose enc and nrp into [d, s] (bf16) ----
    with tc.tile_pool(name="trin", bufs=2) as trin, \
         tc.tile_pool(name="trps", bufs=1, space="PSUM") as trps:
        for src, dst in ((enc, encT), (nrp, nrpT)):
            for tq in range(0, ST, 4):
                ptiles = [trps.tile([P, 4 * P], BF16, tag=f"tr{c}", name=f"tr{c}") for c in range(DCH)]
                for t in range(tq, tq + 4):
                    s_in = trin.tile([P, D_MODEL], BF16, tag="s_in", name="s_in")
                    nc.sync.dma_start(out=s_in, in_=src[t * P:(t + 1) * P, :])
                    for c in range(DCH):
                        nc.tensor.transpose(
                            ptiles[c][:, (t - tq) * P:(t - tq + 1) * P],
                            s_in[:, c * P:(c + 1) * P],
                            ident_bf,
                        )
                for c in range(DCH):
                    nc.any.tensor_copy(dst[c][:, tq * P:(tq + 4) * P], ptiles[c])

    qt_pool = tc.alloc_tile_pool(name="qt", bufs=1)
    kt_pool = tc.alloc_tile_pool(name="kt", bufs=1)
    qt = [qt_pool.tile([P, SEQ], BF16, tag=f"qt{g}", name=f"qt{g}") for g in range(PAIRS)]
    kt = [kt_pool.tile([P, SEQ], BF16, tag=f"kt{g}", name=f"kt{g}") for g in range(PAIRS)]
    va_pool = tc.alloc_tile_pool(name="va", bufs=1)
    va = [va_pool.tile([P, VTOT], BF16, tag=f"va{t}", name=f"va{t}") for t in range(ST)]
    zt_pool = tc.alloc_tile_pool(name="zt", bufs=1)
    zt = [zt_pool.tile([P, SEQ], BF16, tag=f"zt{k}", name=f"zt{k}") for k in range(DCH)]

    # q/k projection for one pair = 4 PSUM groups of 8 chunk-matmuls
    pproj = tc.alloc_tile_pool(name="pproj", bufs=2, space="PSUM")

    def proj_group_mms(g, w, bcol, dst, n0):
        pp = pproj.tile([P, 512], F32, tag="pp", name="pp")
        for c in range(DCH):
            nc.tensor.matmul(
                pp,
                w[c][:, g * P:(g + 1) * P],
                encT[c][:, n0:n0 + 512],
                start=(c == 0), stop=(c == DCH - 1),
                skip_group_check=True,
            )
        nc.vector.tensor_scalar_add(
            out=dst[g][:, n0:n0 + 512], in0=pp, scalar1=bcol[:, g:g + 1],
        )

    def proj_pair_units(g):
        # 4 callables, each emitting one accumulation group (8 PE matmuls)
        units = []
        for w, bcol, dst in ((qw, qb_col, qt), (kw, kb_col, kt)):
            for n0 in (0, 512):
                units.append(lambda g=g, w=w, bcol=bcol, dst=dst, n0=n0:
                             proj_group_mms(g, w, bcol, dst, n0))
        return units

    # ---- phase 2: v projection -> va [m, 16*65] with ones columns,
    #      with q/k projections for pairs 0 and 1 interleaved ----
    with tc.tile_pool(name="pv", bufs=2, space="PSUM") as pv:
        vgroups = proj_pair_units(0) + proj_pair_units(1)
        for t in range(ST):
            pt = pv.tile([P, VTOT], F32, tag="pv", name="pvt")
            for c in range(DCH):
                for n0 in range(0, VTOT, 512):
                    nw = min(512, VTOT - n0)
                    nc.tensor.matmul(
                        pt[:, n0:n0 + nw],
                        nrpT[c][:, t * P:(t + 1) * P],
                        vw[c][:, n0:n0 + nw],
                        start=(c == 0), stop=(c == DCH - 1),
                        skip_group_check=True,
                    )
            vgroups[t]()
            # vb_bc has the per-(h,dh) bias, with 1.0 in each ones-column slot;
            # matmul wrote 0 there (vst ones-columns are zero), so add gives 1.0
            nc.vector.tensor_add(va[t], pt, vb_bc)

    nrp_t_pool.release()

    # ---- phase 3: attention; pair g+1's q/k projection and pair g-1's
    #      softmax normalization ride the PE filler queue ----
    with tc.tile_pool(name="attn", bufs=3) as apool, \
         tc.tile_pool(name="rnorm", bufs=1) as rpool, \
         tc.tile_pool(name="ps_s", bufs=2, space="PSUM") as spool, \
         tc.tile_pool(name="ps_z", bufs=1, space="PSUM") as zpool:

        def norm_pair_units(pg):
            # reciprocal of softmax sums for pair pg, broadcast + apply to zt.
            # sums [2, 1024] -> [16, 128] so the FD-bound reciprocal runs
            # across partitions; back via DRAM to [2, 1024] for the K=2
            # selector broadcast matmul.
            s2 = rpool.tile([N_HEADS, P], F32, tag="s2", name="s2", bufs=2)
            nc.sync.dma_start(
                out=s2,
                in_=sums_dram[2 * pg:2 * pg + 2, :].rearrange("h (a c) -> (h a) c", c=P),
            )
            r2x = rpool.tile([N_HEADS, P], F32R, tag="r2x", name="r2x", bufs=2)
            with nc.allow_low_precision(reason="softmax denominators are O(1); fp32r rounding is fine"):
                nc.vector.reciprocal(out=r2x, in_=s2)
            nc.sync.dma_start(out=rcp_dram[pg], in_=r2x)
            r2 = rpool.tile([2, SEQ], F32R, tag="r2", name="r2", bufs=2)
            nc.sync.dma_start(out=r2, in_=rcp_dram[pg].rearrange("(h a) c -> h (a c)", h=2))

            def apply(n0, r2=r2, pg=pg):
                pb = pproj.tile([P, 512], F32, tag="pp", name="ppb")
                nc.tensor.matmul(
                    pb, sel2, r2[:, n0:n0 + 512],
                    start=True, stop=True, skip_group_check=True,
                )
                nc.vector.tensor_mul(zt[pg][:, n0:n0 + 512], zt[pg][:, n0:n0 + 512], pb)

            return [lambda n0=n0: apply(n0) for n0 in (0, 512)]

        for h in range(N_HEADS):
            g, off = h // 2, (h % 2) * D_HEAD
            filler = []
            if h % 2 == 0:
                if g >= 1:
                    filler += norm_pair_units(g - 1)
                if 2 <= g + 2 < PAIRS:
                    filler += proj_pair_units(g + 2)[:2]
            else:
                if 2 <= g + 2 < PAIRS:
                    filler += proj_pair_units(g + 2)[2:]
            pz = zpool.tile([VW, SEQ], F32, tag="pz", name="pz")

            def av_mms(i, ae):
                q0 = i * P
                for n0, nw in _bank_splits(q0):
                    nc.tensor.matmul(
                        pz[:, n0:n0 + nw],
                        va[i][:, h * VW:(h + 1) * VW],
                        ae[:, n0:n0 + nw],
                        start=(i == 0), stop=(i == ST - 1),
                        skip_group_check=True,
                    )

            pend = None
            for i in range(ST):
                q0 = i * P
                ps = spool.tile([P, SEQ], F32, tag="ps", name="ps")
                ae = apool.tile([P, SEQ], BF16, tag="ae", name="ae")
                for n0, nw in _bank_splits(q0):
                    nc.tensor.matmul(
                        ps[:, n0:n0 + nw],
                        kt[g][off:off + D_HEAD, q0:q0 + P],
                        qt[g][off:off + D_HEAD, n0:n0 + nw],
                        start=True, stop=(n0 != q0),
                        skip_group_check=True,
                    )
                # causal diag mask: accumulate I.T @ M0
                nc.tensor.matmul(
                    ps[:, q0:q0 + P],
                    ident_bf, mask_bf,
                    start=False, stop=True,
                    skip_group_check=True,
                )
                if filler and i % 2 == 0:
                    filler.pop(0)()
                nc.scalar.activation(
                    out=ae[:, q0:SEQ], in_=ps[:, q0:SEQ],
                    func=AF.Exp, scale=float(SCALE),
                )
                # attn@v delayed one chunk so exp latency hides behind PE work
                if pend is not None:
                    av_mms(*pend)
                pend = (i, ae)
            av_mms(*pend)
            for u in filler:
                u()
            # stash unnormalized zT and the denominator row; frees the PSUM slot
            nc.vector.tensor_copy(zt[g][off:off + D_HEAD, :], pz[0:D_HEAD, :])
            srow = rpool.tile([1, SEQ], F32, tag="srow", name="srow", bufs=2)
            nc.vector.tensor_copy(srow, pz[D_HEAD:VW, :])
            nc.sync.dma_start(out=sums_dram[h:h + 1, :], in_=srow)

        for u in norm_pair_units(PAIRS - 1):
            u()

    pproj.release()

    # ---- phase 4: output projection out[s, d] = zt.T @ O + ob ----
    with tc.tile_pool(name="outsb", bufs=3) as outsb, \
         tc.tile_pool(name="po", bufs=2, space="PSUM") as po:
        for t in range(ST):
            pt = po.tile([P, D_MODEL], F32, tag="po", name="pot")
            for k in range(DCH):
                for n0 in range(0, D_MODEL, 512):
                    nc.tensor.matmul(
                        pt[:, n0:n0 + 512],
                        zt[k][:, t * P:(t + 1) * P],
                        owt[k][:, n0:n0 + 512],
                        start=(k == 0), stop=(k == DCH - 1),
                    )
            ot = outsb.tile([P, D_MODEL], F32, tag="ot", name="ot")
            nc.vector.tensor_add(ot, pt, ob_bc)
            nc.sync.dma_start(out=out[t * P:(t + 1) * P, :], in_=ot)

    for pool in (zt_pool, va_pool, kt_pool, qt_pool, enc_t_pool, wpool, smalls):
        pool.release()


def _get_program():
    if "nc" not in _CACHE:
        _CACHE["nc"] = _build_program()
    return _CACHE["nc"]


def _pack_weights(Qs, Qbs, Ks, Kbs, Vs, Vbs, O, Ob):
    f = np.float32
    qst = np.ascontiguousarray(np.transpose(np.asarray(Qs, f), (1, 0, 2)).reshape(D_MODEL, D_MODEL)).astype(NPBF16)
    kst = np.ascontiguousarray(np.transpose(np.asarray(Ks, f), (1, 0, 2)).reshape(D_MODEL, D_MODEL)).astype(NPBF16)
    vst = np.zeros((D_MODEL, VTOT), f)
    vb = np.zeros((VTOT,), f)
    Vs = np.asarray(Vs, f)
    Vbs = np.asarray(Vbs, f)
    for h in range(N_HEADS):
        vst[:, h * VW:h * VW + D_HEAD] = Vs[h]
        vb[h * VW:h * VW + D_HEAD] = Vbs[h]
        vb[h * VW + D_HEAD] = 1.0
    vst = vst.astype(NPBF16)
    ow = np.ascontiguousarray(np.asarray(O, f).reshape(D_MODEL, D_MODEL)).astype(NPBF16)
    qbf = np.ascontiguousarray(np.asarray(Qbs, f).reshape(D_MODEL))
    kbf = np.ascontiguousarray(np.asarray(Kbs, f).reshape(D_MODEL))
    obf = np.ascontiguousarray(np.asarray(Ob, f).reshape(D_MODEL))
    return qst, kst, vst, ow, qbf, kbf, vb, obf


def kernel(normalized_resid_pre, encoder_output, Qs, Qbs, Ks, Kbs, Vs, Vbs, O, Ob,
           _trace=False, _trace_kwargs=None):
    nc = _get_program()
    qst, kst, vst, ow, qbf, kbf, vb, obf = _pack_weights(Qs, Qbs, Ks, Kbs, Vs, Vbs, O, Ob)
    enc = np.asarray(encoder_output, np.float32).astype(NPBF16)
    nrp = np.asarray(normalized_resid_pre, np.float32).astype(NPBF16)
    in_maps = []
    for b in range(BATCH):
        in_maps.append({
            "enc": np.ascontiguousarray(enc[b]),
            "nrp": np.ascontiguousarray(nrp[b]),
            "qst": qst, "kst": kst, "vst": vst, "ow": ow,
            "qb": qbf, "kb": kbf, "vb": vb, "ob": obf,
        })
    res = run_bass_kernel_spmd(
        nc, in_maps, list(range(BATCH)),
        trace=_trace, **(_trace_kwargs or {}),
    )
    out = np.stack([res.results[b]["out"] for b in range(BATCH)], axis=0)
    if _trace:
        _CACHE["last_results"] = res
    return out


# revision 5
# speedup vs baseline: 1.3473x; 1.0421x over previous
"""DecoderAttention Bass/Tile kernel for TRN2, batch-parallel over 8 NeuronCores.

Each core handles one batch element:
  q = enc @ Qs + Qbs ; k = enc @ Ks + Kbs ; v = nrp @ Vs + Vbs   (per head)
  scores = q k^T / sqrt(64), causal mask (-1e5), softmax
  out = (attn @ v) @ O + Ob

Layout/throughput strategy (all matmuls in bf16 at 1 col/cycle, f32 PSUM):
  - all big DRAM inputs pre-cast to bf16 host-side (halves DMA, enables
    bf16 PE transposes and 2x matmul rate vs fp32r; measured end-to-end
    max rel err ~4e-3 vs the 2e-2 gate)
  - enc/nrp transposed on-device (PE transpose, bf16 PSUM) to [d, s];
    nrp first so the v projection can start as early as possible
  - a burst of dummy matmuls at t=0 holds the PE busy through one full
    HAM activity window, lifting the clock gate to 2.4 GHz before the
    DMA-paced transpose phase (which alone leaves gaps that keep it cold)
  - weights pre-packed host-side to [d, (h dh)]; Vs padded to [d, 16*65]
    with a ones column per head so attn@v also produces softmax row sums
  - scoresT [m, q] per head so exp output feeds attn@v without transposing
  - causal diagonal blocks masked by accumulating I.T @ M0 (bf16) in PSUM
  - exp folds the 1/sqrt(d_head) scale; no max subtraction (scores are O(1),
    masked entries become exactly 0)
  - HAM needs gapless PE activity to hold 2.4 GHz, so independent PE work
    rides a filler queue threaded through the v-projection and attention
    loops: q/k projections for later pairs, and the softmax-normalization
    broadcast matmuls of earlier pairs (marked late so the PE never
    blocks on their reciprocal chain)
  - softmax division deferred: per-pair reciprocal over sums reshaped to
    [16, 128] (partition-parallel reciprocal), broadcast back across
    partitions with 8 one-hot selector matmuls (no DRAM reshape on the
    read side), applied to zt while later pairs run
"""

import numpy as np
import ml_dtypes

import concourse.bass as bass
import concourse.mybir as mybir
import concourse.tile as tile
from concourse import bacc
from concourse.bass_utils import run_bass_kernel_spmd
from concourse.masks import make_identity

N_HEADS, D_MODEL, D_HEAD = 16, 1024, 64
BATCH, SEQ = 8, 1024
P = 128
DCH = D_MODEL // P       # 8 contraction chunks
ST = SEQ // P            # 8 seq tiles
PAIRS = N_HEADS // 2     # 8 head pairs
VW = 65                  # v width per head incl. ones column
VTOT = N_HEADS * VW      # 1040
IGNORE = -100000.0
SCALE = 1.0 / np.sqrt(np.float32(D_HEAD))

F32 = mybir.dt.float32
F32R = mybir.dt.float32r
BF16 = mybir.dt.bfloat16
AF = mybir.ActivationFunctionType
NPBF16 = ml_dtypes.bfloat16

_CACHE = {}


def _bank_splits(q0):
    # PSUM-bank-aligned (n0, nw) column splits covering [q0, SEQ)
    if q0 < 512:
        return [(q0, 512 - q0), (512, 512)]
    return [(q0, SEQ - q0)]


def _bcast_row_ap(src, n):
    # DMA access pattern replicating a [n]-element DRAM row to 128 partitions
    return bass.AP(tensor=src.tensor, offset=src.offset, ap=[[0, P], [1, n]])


def _build_program():
    nc = bacc.Bacc("TRN2", target_bir_lowering=False, debug=False, num_devices=8)

    enc = nc.dram_tensor("enc", [SEQ, D_MODEL], BF16, kind="ExternalInput").ap()
    nrp = nc.dram_tensor("nrp", [SEQ, D_MODEL], BF16, kind="ExternalInput").ap()
    qst = nc.dram_tensor("qst", [D_MODEL, D_MODEL], BF16, kind="ExternalInput").ap()
    kst = nc.dram_tensor("kst", [D_MODEL, D_MODEL], BF16, kind="ExternalInput").ap()
    vst = nc.dram_tensor("vst", [D_MODEL, VTOT], BF16, kind="ExternalInput").ap()
    ow = nc.dram_tensor("ow", [D_MODEL, D_MODEL], BF16, kind="ExternalInput").ap()
    qb = nc.dram_tensor("qb", [D_MODEL], F32, kind="ExternalInput").ap()
    kb = nc.dram_tensor("kb", [D_MODEL], F32, kind="ExternalInput").ap()
    vb = nc.dram_tensor("vb", [VTOT], F32, kind="ExternalInput").ap()
    ob = nc.dram_tensor("ob", [D_MODEL], F32, kind="ExternalInput").ap()
    out = nc.dram_tensor("out", [SEQ, D_MODEL], F32, kind="ExternalOutput").ap()
    sums_dram = nc.dram_tensor("sums_scratch", [N_HEADS, SEQ], F32).ap()

    with tile.TileContext(nc) as tc:
        _kernel(tc, out, enc, nrp, qst, kst, vst, ow, qb, kb, vb, ob,
                sums_dram=sums_dram)
    nc.compile()
    return nc


def _kernel(tc, out, enc, nrp, qst, kst, vst, ow, qb, kb, vb, ob, sums_dram=None):
    nc = tc.nc

    smalls = tc.alloc_tile_pool(name="smalls", bufs=1)
    ident_bf = smalls.tile([P, P], BF16, tag="ident_bf", name="ident_bf")
    make_identity(nc, ident_bf)
    # M0[m, q] = IGNORE where m > q else 0 (strict causal mask, diag block)
    mask_bf = smalls.tile([P, P], BF16, tag="mask_bf", name="mask_bf")
    nc.gpsimd.memset(mask_bf, 0.0)
    nc.gpsimd.affine_select(
        out=mask_bf, in_=mask_bf,
        compare_op=mybir.AluOpType.is_ge,
        fill=IGNORE, base=0,
        pattern=[[1, P]], channel_multiplier=-1,
    )
    # sel[a][j, p] = 1 where j == (p // 64) * 8 + a: broadcasts the [16, 128]
    # reciprocal layout (row = (head, seg), col = q%128) to [128, q-seg a]
    sel = []
    self_f = smalls.tile([N_HEADS, P], F32, tag="self", name="self")
    for a in range(DCH):
        nc.gpsimd.memset(self_f, 0.0)
        nc.gpsimd.affine_select(
            out=self_f.rearrange("j (h c) -> j h c", h=2),
            in_=self_f.rearrange("j (h c) -> j h c", h=2),
            compare_op=mybir.AluOpType.not_equal,
            fill=1.0, base=-a,
            pattern=[[-8, 2], [0, D_HEAD]], channel_multiplier=1,
        )
        s_r = smalls.tile([N_HEADS, P], F32R, tag=f"sel{a}", name=f"sel{a}")
        nc.vector.tensor_copy(s_r, self_f)
        sel.append(s_r)
    vb_bc = smalls.tile([P, VTOT], F32, tag="vb_bc", name="vb_bc")
    nc.sync.dma_start(out=vb_bc, in_=_bcast_row_ap(vb, VTOT))
    ob_bc = smalls.tile([P, D_MODEL], F32, tag="ob_bc", name="ob_bc")
    nc.sync.dma_start(out=ob_bc, in_=_bcast_row_ap(ob, D_MODEL))
    qb_col = smalls.tile([P, PAIRS], F32, tag="qb_col", name="qb_col")
    nc.sync.dma_start(out=qb_col, in_=qb.rearrange("(g p) -> p g", p=P))
    kb_col = smalls.tile([P, PAIRS], F32, tag="kb_col", name="kb_col")
    nc.sync.dma_start(out=kb_col, in_=kb.rearrange("(g p) -> p g", p=P))

    # persistent weight tiles (bf16), prefetched on the scalar DMA queue in
    # consumption order (v projection first, output projection last)
    wpool = tc.alloc_tile_pool(name="weights", bufs=1, side="right")
    vw = [wpool.tile([P, VTOT], BF16, tag=f"vw{c}", name=f"vw{c}") for c in range(DCH)]
    qw = [wpool.tile([P, D_MODEL], BF16, tag=f"qw{c}", name=f"qw{c}") for c in range(DCH)]
    kw = [wpool.tile([P, D_MODEL], BF16, tag=f"kw{c}", name=f"kw{c}") for c in range(DCH)]
    owt = [wpool.tile([P, D_MODEL], BF16, tag=f"owt{c}", name=f"owt{c}") for c in range(DCH)]
    for c in range(DCH):
        nc.scalar.dma_start(out=vw[c], in_=vst[c * P:(c + 1) * P, :])
    for c in range(DCH):
        nc.scalar.dma_start(out=qw[c], in_=qst[c * P:(c + 1) * P, :])
    for c in range(DCH):
        nc.scalar.dma_start(out=kw[c], in_=kst[c * P:(c + 1) * P, :])
    for c in range(DCH):
        nc.scalar.dma_start(out=owt[c], in_=ow[c * P:(c + 1) * P, :])

    enc_t_pool = tc.alloc_tile_pool(name="encT", bufs=1, side="right")
    nrp_t_pool = tc.alloc_tile_pool(name="nrpT", bufs=1, side="right")
    encT = [enc_t_pool.tile([P, SEQ], BF16, tag=f"encT{c}", name=f"encT{c}") for c in range(DCH)]
    nrpT = [nrp_t_pool.tile([P, SEQ], BF16, tag=f"nrpT{c}", name=f"nrpT{c}") for c in range(DCH)]

    # ---- phase 1: transpose nrp then enc into [d, s] (bf16), after a
    #      gapless dummy-matmul burst that lifts the HAM clock gate ----
    with tc.tile_pool(name="warmps", bufs=1, space="PSUM") as wps:
        warm = wps.tile([P, P], F32, tag="warm", name="warm")
        for _ in range(32):
            nc.tensor.matmul(warm, ident_bf, ident_bf, start=True, stop=True,
                             skip_group_check=True)
    with tc.tile_pool(name="trin", bufs=3) as trin, \
         tc.tile_pool(name="trps", bufs=1, space="PSUM") as trps:
        for src, dst in ((nrp, nrpT), (enc, encT)):
            for tq in range(0, ST, 4):
                ptiles = [trps.tile([P, 4 * P], BF16, tag=f"tr{c}", name=f"tr{c}") for c in range(DCH)]
                for t in range(tq, tq + 4):
                    s_in = trin.tile([P, D_MODEL], BF16, tag="s_in", name="s_in")
                    nc.sync.dma_start(out=s_in, in_=src[t * P:(t + 1) * P, :])
                    for c in range(DCH):
                        nc.tensor.transpose(
                            ptiles[c][:, (t - tq) * P:(t - tq + 1) * P],
                            s_in[:, c * P:(c + 1) * P],
                            ident_bf,
                        )
                for c in range(DCH):
                    nc.any.tensor_copy(dst[c][:, tq * P:(tq + 4) * P], ptiles[c])

    qt_pool = tc.alloc_tile_pool(name="qt", bufs=1)
    kt_pool = tc.alloc_tile_pool(name="kt", bufs=1)
    qt = [qt_pool.tile([P, SEQ], BF16, tag=f"qt{g}", name=f"qt{g}") for g in range(PAIRS)]
    kt = [kt_pool.tile([P, SEQ], BF16, tag=f"kt{g}", name=f"kt{g}") for g in range(PAIRS)]
    va_pool = tc.alloc_tile_pool(name="va", bufs=1)
    va = [va_pool.tile([P, VTOT], BF16, tag=f"va{t}", name=f"va{t}") for t in range(ST)]
    zt_pool = tc.alloc_tile_pool(name="zt", bufs=1)
    zt = [zt_pool.tile([P, SEQ], BF16, tag=f"zt{k}", name=f"zt{k}") for k in range(DCH)]

    pproj = tc.alloc_tile_pool(name="pproj", bufs=2, space="PSUM")
    rpool = tc.alloc_tile_pool(name="rnorm", bufs=1)

    # ---- PE filler queue: (early_ok, emit). Early slots sit right behind
    # fresh attention matmuls, so only latency-free work may go there. ----
    filler = []

    def proj_group_mms(g, w, bcol, dst, n0):
        pp = pproj.tile([P, 512], F32, tag="pp", name="pp")
        for c in range(DCH):
            nc.tensor.matmul(
                pp,
                w[c][:, g * P:(g + 1) * P],
                encT[c][:, n0:n0 + 512],
                start=(c == 0), stop=(c == DCH - 1),
                skip_group_check=True,
            )
        nc.vector.tensor_scalar_add(
            out=dst[g][:, n0:n0 + 512], in0=pp, scalar1=bcol[:, g:g + 1],
        )

    def push_proj_pair(g):
        for w, bcol, dst in ((qw, qb_col, qt), (kw, kb_col, kt)):
            for n0 in (0, 512):
                filler.append((True, lambda g=g, w=w, bcol=bcol, dst=dst, n0=n0:
                               proj_group_mms(g, w, bcol, dst, n0)))

    def push_norm_pair(pg):
        # reciprocal of softmax sums for pair pg: gather the two sum rows as
        # [16, 128] so the FD-bound reciprocal runs across partitions, then
        # broadcast to [128, SEQ] via the one-hot selector matmuls.
        s2 = rpool.tile([N_HEADS, P], F32, tag="s2", name="s2", bufs=2)
        nc.sync.dma_start(
            out=s2,
            in_=sums_dram[2 * pg:2 * pg + 2, :].rearrange("h (a c) -> (h a) c", c=P),
        )
        r2 = rpool.tile([N_HEADS, P], F32R, tag="r2", name="r2", bufs=2)
        with nc.allow_low_precision(reason="softmax denominators are O(1); fp32r rounding is fine"):
            nc.vector.reciprocal(out=r2, in_=s2)

        def apply(half, r2=r2, pg=pg):
            pb = pproj.tile([P, 512], F32, tag="pp", name="ppb")
            for a in range(4 * half, 4 * half + 4):
                nc.tensor.matmul(
                    pb[:, (a % 4) * P:(a % 4 + 1) * P], sel[a], r2,
                    start=True, stop=True, skip_group_check=True,
                )
            nc.vector.tensor_mul(
                zt[pg][:, half * 512:half * 512 + 512],
                zt[pg][:, half * 512:half * 512 + 512],
                pb,
            )

        for half in (0, 1):
            filler.append((False, lambda half=half: apply(half)))

    def pop_filler(allow_late):
        for idx, (early_ok, emit) in enumerate(filler):
            if early_ok or allow_late:
                filler.pop(idx)
                emit()
                return

    # ---- phase 2: v projection -> va [m, 16*65] with ones columns,
    #      with q/k projections for pairs 0 and 1 riding the filler queue ----
    push_proj_pair(0)
    push_proj_pair(1)
    with tc.tile_pool(name="pv", bufs=2, space="PSUM") as pv:
        for t in range(ST):
            pt = pv.tile([P, VTOT], F32, tag="pv", name="pvt")
            for c in range(DCH):
                for n0 in range(0, VTOT, 512):
                    nw = min(512, VTOT - n0)
                    nc.tensor.matmul(
                        pt[:, n0:n0 + nw],
                        nrpT[c][:, t * P:(t + 1) * P],
                        vw[c][:, n0:n0 + nw],
                        start=(c == 0), stop=(c == DCH - 1),
                        skip_group_check=True,
                    )
            if t >= 1:
                pop_filler(allow_late=True)
            # vb_bc has the per-(h,dh) bias, with 1.0 in each ones-column slot;
            # matmul wrote 0 there (vst ones-columns are zero), so add gives 1.0
            nc.vector.tensor_add(va[t], pt, vb_bc)

    nrp_t_pool.release()

    # ---- phase 3: attention; later pairs' q/k projections and earlier
    #      pairs' softmax normalization ride the PE filler queue ----
    with tc.tile_pool(name="attn", bufs=3) as apool, \
         tc.tile_pool(name="ps_s", bufs=2, space="PSUM") as spool, \
         tc.tile_pool(name="ps_z", bufs=1, space="PSUM") as zpool:
        for h in range(N_HEADS):
            g, off = h // 2, (h % 2) * D_HEAD
            if h % 2 == 0:
                if g >= 1:
                    push_norm_pair(g - 1)
                if g + 2 < PAIRS:
                    push_proj_pair(g + 2)
            pz = zpool.tile([VW, SEQ], F32, tag="pz", name="pz")

            def av_mms(i, ae):
                q0 = i * P
                for n0, nw in _bank_splits(q0):
                    nc.tensor.matmul(
                        pz[:, n0:n0 + nw],
                        va[i][:, h * VW:(h + 1) * VW],
                        ae[:, n0:n0 + nw],
                        start=(i == 0), stop=(i == ST - 1),
                        skip_group_check=True,
                    )

            pend = None
            for i in range(ST):
                q0 = i * P
                ps = spool.tile([P, SEQ], F32, tag="ps", name="ps")
                ae = apool.tile([P, SEQ], BF16, tag="ae", name="ae")
                for n0, nw in _bank_splits(q0):
                    nc.tensor.matmul(
                        ps[:, n0:n0 + nw],
                        kt[g][off:off + D_HEAD, q0:q0 + P],
                        qt[g][off:off + D_HEAD, n0:n0 + nw],
                        start=True, stop=(n0 != q0),
                        skip_group_check=True,
                    )
                # causal diag mask: accumulate I.T @ M0
                nc.tensor.matmul(
                    ps[:, q0:q0 + P],
                    ident_bf, mask_bf,
                    start=False, stop=True,
                    skip_group_check=True,
                )
                if i % 2 == 0:
                    pop_filler(allow_late=(i >= 4))
                nc.scalar.activation(
                    out=ae[:, q0:SEQ], in_=ps[:, q0:SEQ],
                    func=AF.Exp, scale=float(SCALE),
                )
                # attn@v delayed one chunk so exp latency hides behind PE work
                if pend is not None:
                    av_mms(*pend)
                pend = (i, ae)
            av_mms(*pend)
            # stash unnormalized zT and the denominator row; frees the PSUM slot
            nc.vector.tensor_copy(zt[g][off:off + D_HEAD, :], pz[0:D_HEAD, :])
            srow = rpool.tile([1, SEQ], F32, tag="srow", name="srow", bufs=2)
            nc.vector.tensor_copy(srow, pz[D_HEAD:VW, :])
            nc.sync.dma_start(out=sums_dram[h:h + 1, :], in_=srow)

        push_norm_pair(PAIRS - 1)
        while filler:
            pop_filler(allow_late=True)

    pproj.release()

    # ---- phase 4: output projection out[s, d] = zt.T @ O + ob ----
    with tc.tile_pool(name="outsb", bufs=3) as outsb, \
         tc.tile_pool(name="po", bufs=2, space="PSUM") as po:
        for t in range(ST):
            pt = po.tile([P, D_MODEL], F32, tag="po", name="pot")
            for k in range(DCH):
                for n0 in range(0, D_MODEL, 512):
                    nc.tensor.matmul(
                        pt[:, n0:n0 + 512],
                        zt[k][:, t * P:(t + 1) * P],
                        owt[k][:, n0:n0 + 512],
                        start=(k == 0), stop=(k == DCH - 1),
                    )
            ot = outsb.tile([P, D_MODEL], F32, tag="ot", name="ot")
            nc.vector.tensor_add(ot, pt, ob_bc)
            nc.sync.dma_start(out=out[t * P:(t + 1) * P, :], in_=ot)

    for pool in (rpool, zt_pool, va_pool, kt_pool, qt_pool, enc_t_pool, wpool, smalls):
        pool.release()


def _get_program():
    if "nc" not in _CACHE:
        _CACHE["nc"] = _build_program()
    return _CACHE["nc"]


def _pack_weights(Qs, Qbs, Ks, Kbs, Vs, Vbs, O, Ob):
    f = np.float32
    qst = np.ascontiguousarray(np.transpose(np.asarray(Qs, f), (1, 0, 2)).reshape(D_MODEL, D_MODEL)).astype(NPBF16)
    kst = np.ascontiguousarray(np.transpose(np.asarray(Ks, f), (1, 0, 2)).reshape(D_MODEL, D_MODEL)).astype(NPBF16)
    vst = np.zeros((D_MODEL, VTOT), f)
    vb = np.zeros((VTOT,), f)
    Vs = np.asarray(Vs, f)
    Vbs = np.asarray(Vbs, f)
    for h in range(N_HEADS):
        vst[:, h * VW:h * VW + D_HEAD] = Vs[h]
        vb[h * VW:h * VW + D_HEAD] = Vbs[h]
        vb[h * VW + D_HEAD] = 1.0
    vst = vst.astype(NPBF16)
    ow = np.ascontiguousarray(np.asarray(O, f).reshape(D_MODEL, D_MODEL)).astype(NPBF16)
    qbf = np.ascontiguousarray(np.asarray(Qbs, f).reshape(D_MODEL))
    kbf = np.ascontiguousarray(np.asarray(Kbs, f).reshape(D_MODEL))
    obf = np.ascontiguousarray(np.asarray(Ob, f).reshape(D_MODEL))
    return qst, kst, vst, ow, qbf, kbf, vb, obf


def kernel(normalized_resid_pre, encoder_output, Qs, Qbs, Ks, Kbs, Vs, Vbs, O, Ob,
           _trace=False, _trace_kwargs=None):
    nc = _get_program()
    qst, kst, vst, ow, qbf, kbf, vb, obf = _pack_weights(Qs, Qbs, Ks, Kbs, Vs, Vbs, O, Ob)
    enc = np.asarray(encoder_output, np.float32).astype(NPBF16)
    nrp = np.asarray(normalized_resid_pre, np.float32).astype(NPBF16)
    in_maps = []
    for b in range(BATCH):
        in_maps.append({
            "enc": np.ascontiguousarray(enc[b]),
            "nrp": np.ascontiguousarray(nrp[b]),
            "qst": qst, "kst": kst, "vst": vst, "ow": ow,
            "qb": qbf, "kb": kbf, "vb": vb, "ob": obf,
        })
    res = run_bass_kernel_spmd(
        nc, in_maps, list(range(BATCH)),
        trace=_trace, **(_trace_kwargs or {}),
    )
    out = np.stack([res.results[b]["out"] for b in range(BATCH)], axis=0)
    if _trace:
        _CACHE["last_results"] = res
    return out


# revision 14
# speedup vs baseline: 1.4733x; 1.0935x over previous
"""DecoderAttention Bass/Tile kernel for TRN2, batch-parallel over 8 NeuronCores.

Each core handles one batch element:
  q = enc @ Qs + Qbs ; k = enc @ Ks + Kbs ; v = nrp @ Vs + Vbs   (per head)
  scores = q k^T / sqrt(64), causal mask (-1e5), softmax
  out = (attn @ v) @ O + Ob

Layout/throughput strategy (all matmuls in bf16 at 1 col/cycle, f32 PSUM):
  - all big DRAM inputs pre-cast to bf16 host-side (halves DMA, enables
    bf16 PE transposes and 2x matmul rate vs fp32r; measured end-to-end
    max rel err ~4e-3 vs the 2e-2 gate)
  - enc/nrp transposed on-device (PE transpose, bf16 PSUM) to [d, s];
    nrp first so the v projection can start as early as possible; each
    seq-tile load is split across the sync and vector DMA queues (one
    queue sustains only ~150 GB/s on 2KB-per-partition patterns), and
    bias/broadcast DMAs ride the gpsimd queue so they never head-of-line
    block the transpose feed
  - a burst of dummy matmuls at t=0 holds the PE busy through one full
    HAM activity window, lifting the clock gate to 2.4 GHz early
  - weights pre-packed host-side to [d, (h dh)]; Vs padded to [d, 16*65]
    with a ones column per head so attn@v also produces softmax row sums
  - scoresT [m, q] per head so exp output feeds attn@v without transposing
  - causal diagonal blocks masked by accumulating I.T @ M0 (bf16) in PSUM
  - exp folds the 1/sqrt(d_head) scale; no max subtraction (scores are O(1),
    masked entries become exactly 0)
  - the HAM clock gate re-throttles whenever PE duty sags inside a ~3.4us
    window, so independent PE work rides a fine-grained filler queue
    threaded through the v-projection and attention loops: q/k projection
    half-groups (4 matmuls) for later pairs, and the softmax-normalization
    broadcast matmuls of earlier pairs (marked late so the PE never blocks
    on their reciprocal chain); units are pair-tagged and force-drained
    before the pair that needs them
  - softmax division deferred: per-pair reciprocal over sums reshaped to
    [16, 128] (partition-parallel reciprocal), broadcast back across
    partitions with 8 one-hot selector matmuls, applied to zt while later
    pairs run; the final pair's normalization overlaps the first output-
    projection accumulations (their k=7 contribution is issued last)
"""

import numpy as np
import ml_dtypes

import concourse.bass as bass
import concourse.mybir as mybir
import concourse.tile as tile
from concourse import bacc
from concourse.bass_utils import run_bass_kernel_spmd
from concourse.masks import make_identity

N_HEADS, D_MODEL, D_HEAD = 16, 1024, 64
BATCH, SEQ = 8, 1024
P = 128
DCH = D_MODEL // P       # 8 contraction chunks
ST = SEQ // P            # 8 seq tiles
PAIRS = N_HEADS // 2     # 8 head pairs
VW = 65                  # v width per head incl. ones column
VTOT = N_HEADS * VW      # 1040
IGNORE = -100000.0
SCALE = 1.0 / np.sqrt(np.float32(D_HEAD))

F32 = mybir.dt.float32
F32R = mybir.dt.float32r
BF16 = mybir.dt.bfloat16
AF = mybir.ActivationFunctionType
NPBF16 = ml_dtypes.bfloat16

_CACHE = {}


def _bank_splits(q0):
    # PSUM-bank-aligned (n0, nw) column splits covering [q0, SEQ)
    if q0 < 512:
        return [(q0, 512 - q0), (512, 512)]
    return [(q0, SEQ - q0)]


def _bcast_row_ap(src, n):
    # DMA access pattern replicating a [n]-element DRAM row to 128 partitions
    return bass.AP(tensor=src.tensor, offset=src.offset, ap=[[0, P], [1, n]])


def _build_program():
    nc = bacc.Bacc("TRN2", target_bir_lowering=False, debug=False, num_devices=8)

    enc = nc.dram_tensor("enc", [SEQ, D_MODEL], BF16, kind="ExternalInput").ap()
    nrp = nc.dram_tensor("nrp", [SEQ, D_MODEL], BF16, kind="ExternalInput").ap()
    qst = nc.dram_tensor("qst", [D_MODEL, D_MODEL], BF16, kind="ExternalInput").ap()
    kst = nc.dram_tensor("kst", [D_MODEL, D_MODEL], BF16, kind="ExternalInput").ap()
    vst = nc.dram_tensor("vst", [D_MODEL, VTOT], BF16, kind="ExternalInput").ap()
    ow = nc.dram_tensor("ow", [D_MODEL, D_MODEL], BF16, kind="ExternalInput").ap()
    # all biases host-packed into one row-replicated [128, 2080] tensor:
    # cols [0:8] qb by pair, [8:16] kb by pair, [16:1056] vb, [1056:2080] ob
    biases = nc.dram_tensor("biases", [P, 16 + VTOT + D_MODEL], F32, kind="ExternalInput").ap()
    out = nc.dram_tensor("out", [SEQ, D_MODEL], F32, kind="ExternalOutput").ap()
    sums_dram = nc.dram_tensor("sums_scratch", [N_HEADS, SEQ], F32).ap()

    with tile.TileContext(nc) as tc:
        _kernel(tc, out, enc, nrp, qst, kst, vst, ow, biases,
                sums_dram=sums_dram)
    nc.compile()
    return nc


def _kernel(tc, out, enc, nrp, qst, kst, vst, ow, biases, sums_dram=None):
    nc = tc.nc

    smalls = tc.alloc_tile_pool(name="smalls", bufs=1)
    ident_bf = smalls.tile([P, P], BF16, tag="ident_bf", name="ident_bf")
    make_identity(nc, ident_bf)
    # M0[m, q] = IGNORE where m > q else 0 (strict causal mask, diag block)
    mask_bf = smalls.tile([P, P], BF16, tag="mask_bf", name="mask_bf")
    nc.gpsimd.memset(mask_bf, 0.0)
    nc.gpsimd.affine_select(
        out=mask_bf, in_=mask_bf,
        compare_op=mybir.AluOpType.is_ge,
        fill=IGNORE, base=0,
        pattern=[[1, P]], channel_multiplier=-1,
    )
    # sel[a][j, p] = 1 where j == (p // 64) * 8 + a: broadcasts the [16, 128]
    # reciprocal layout (row = (head, seg), col = q%128) to [128, q-seg a]
    sel = []
    self_f = smalls.tile([N_HEADS, P], F32, tag="self", name="self")
    for a in range(DCH):
        nc.gpsimd.memset(self_f, 0.0)
        nc.gpsimd.affine_select(
            out=self_f.rearrange("j (h c) -> j h c", h=2),
            in_=self_f.rearrange("j (h c) -> j h c", h=2),
            compare_op=mybir.AluOpType.not_equal,
            fill=1.0, base=-a,
            pattern=[[-8, 2], [0, D_HEAD]], channel_multiplier=1,
        )
        s_r = smalls.tile([N_HEADS, P], F32R, tag=f"sel{a}", name=f"sel{a}")
        nc.vector.tensor_copy(s_r, self_f)
        sel.append(s_r)
    bias_sb = smalls.tile([P, 16 + VTOT + D_MODEL], F32, tag="bias_sb", name="bias_sb")
    qb_col = bias_sb[:, 0:PAIRS]
    kb_col = bias_sb[:, PAIRS:2 * PAIRS]
    vb_bc = bias_sb[:, 16:16 + VTOT]
    ob_bc = bias_sb[:, 16 + VTOT:16 + VTOT + D_MODEL]

    # persistent weight tiles (bf16), prefetched on the scalar DMA queue in
    # consumption order (v projection first, output projection last)
    wpool = tc.alloc_tile_pool(name="weights", bufs=1, side="right")
    vw = [wpool.tile([P, VTOT], BF16, tag=f"vw{c}", name=f"vw{c}") for c in range(DCH)]
    qw = [wpool.tile([P, D_MODEL], BF16, tag=f"qw{c}", name=f"qw{c}") for c in range(DCH)]
    kw = [wpool.tile([P, D_MODEL], BF16, tag=f"kw{c}", name=f"kw{c}") for c in range(DCH)]
    owt = [wpool.tile([P, D_MODEL], BF16, tag=f"owt{c}", name=f"owt{c}") for c in range(DCH)]
    for c in range(DCH):
        nc.scalar.dma_start(out=vw[c], in_=vst[c * P:(c + 1) * P, :])
    for c in range(DCH):
        nc.scalar.dma_start(out=qw[c], in_=qst[c * P:(c + 1) * P, :])
    for c in range(DCH):
        nc.scalar.dma_start(out=kw[c], in_=kst[c * P:(c + 1) * P, :])
    for c in range(DCH):
        nc.scalar.dma_start(out=owt[c], in_=ow[c * P:(c + 1) * P, :])

    enc_t_pool = tc.alloc_tile_pool(name="encT", bufs=1, side="right")
    nrp_t_pool = tc.alloc_tile_pool(name="nrpT", bufs=1, side="right")
    encT = [enc_t_pool.tile([P, SEQ], BF16, tag=f"encT{c}", name=f"encT{c}") for c in range(DCH)]
    nrpT = [nrp_t_pool.tile([P, SEQ], BF16, tag=f"nrpT{c}", name=f"nrpT{c}") for c in range(DCH)]

    # ---- phase 1: transpose nrp then enc into [d, s] (bf16), after a
    #      gapless dummy-matmul burst that lifts the HAM clock gate ----
    with tc.tile_pool(name="warmps", bufs=1, space="PSUM") as wps:
        warm = wps.tile([P, P], F32, tag="warm", name="warm")
        for _ in range(32):
            nc.tensor.matmul(warm, ident_bf, ident_bf, start=True, stop=True,
                             skip_group_check=True)
    with tc.tile_pool(name="trin", bufs=3) as trin, \
         tc.tile_pool(name="trps", bufs=1, space="PSUM") as trps:
        for src, dst in ((nrp, nrpT), (enc, encT)):
            if src is enc:
                nc.sync.dma_start(out=bias_sb, in_=biases)
            for tq in range(0, ST, 4):
                ptiles = [trps.tile([P, 4 * P], BF16, tag=f"tr{c}", name=f"tr{c}") for c in range(DCH)]
                for t in range(tq, tq + 4):
                    s_in = trin.tile([P, D_MODEL], BF16, tag="s_in", name="s_in")
                    nc.sync.dma_start(out=s_in[:, 0:512], in_=src[t * P:(t + 1) * P, 0:512])
                    nc.gpsimd.dma_start(out=s_in[:, 512:D_MODEL], in_=src[t * P:(t + 1) * P, 512:D_MODEL])
                    for c in range(DCH):
                        nc.tensor.transpose(
                            ptiles[c][:, (t - tq) * P:(t - tq + 1) * P],
                            s_in[:, c * P:(c + 1) * P],
                            ident_bf,
                        )
                for c in range(DCH):
                    nc.any.tensor_copy(dst[c][:, tq * P:(tq + 4) * P], ptiles[c])

    qt_pool = tc.alloc_tile_pool(name="qt", bufs=1)
    kt_pool = tc.alloc_tile_pool(name="kt", bufs=1)
    qt = [qt_pool.tile([P, SEQ], BF16, tag=f"qt{g}", name=f"qt{g}") for g in range(PAIRS)]
    kt = [kt_pool.tile([P, SEQ], BF16, tag=f"kt{g}", name=f"kt{g}") for g in range(PAIRS)]
    va_pool = tc.alloc_tile_pool(name="va", bufs=1)
    va = [va_pool.tile([P, VTOT], BF16, tag=f"va{t}", name=f"va{t}") for t in range(ST)]
    zt_pool = tc.alloc_tile_pool(name="zt", bufs=1)
    zt = [zt_pool.tile([P, SEQ], BF16, tag=f"zt{k}", name=f"zt{k}") for k in range(DCH)]

    pproj = tc.alloc_tile_pool(name="pproj", bufs=2, space="PSUM")
    rpool = tc.alloc_tile_pool(name="rnorm", bufs=1)

    # ---- PE filler queue: (pair, early_ok, emit). Early slots sit right
    # behind fresh attention matmuls, so only latency-free work goes there.
    # Units are pair-tagged so everything pair g needs is force-drained
    # before its attention begins. ----
    filler = []
    proj_state = {}

    def proj_half_mms(g, w, bcol, dst, n0, chalf):
        key = (g, id(w), n0)
        if chalf == 0:
            proj_state[key] = pproj.tile([P, 512], F32, tag="pp", name="pp")
        pp = proj_state[key]
        for c in range(4 * chalf, 4 * chalf + 4):
            nc.tensor.matmul(
                pp,
                w[c][:, g * P:(g + 1) * P],
                encT[c][:, n0:n0 + 512],
                start=(c == 0), stop=(c == DCH - 1),
                skip_group_check=True,
            )
        if chalf == 1:
            del proj_state[key]
            nc.vector.tensor_scalar_add(
                out=dst[g][:, n0:n0 + 512], in0=pp, scalar1=bcol[:, g:g + 1],
            )

    def push_proj_pair(g):
        for w, bcol, dst in ((qw, qb_col, qt), (kw, kb_col, kt)):
            for n0 in (0, 512):
                for chalf in (0, 1):
                    filler.append((g, True,
                                   lambda g=g, w=w, bcol=bcol, dst=dst, n0=n0, chalf=chalf:
                                   proj_half_mms(g, w, bcol, dst, n0, chalf)))

    def push_norm_pair(pg):
        # reciprocal of softmax sums for pair pg: gather the two sum rows as
        # [16, 128] so the FD-bound reciprocal runs across partitions, then
        # broadcast to [128, SEQ] via the one-hot selector matmuls.
        s2 = rpool.tile([N_HEADS, P], F32, tag="s2", name="s2", bufs=2)
        nc.sync.dma_start(
            out=s2,
            in_=sums_dram[2 * pg:2 * pg + 2, :].rearrange("h (a c) -> (h a) c", c=P),
        )
        r2 = rpool.tile([N_HEADS, P], F32R, tag="r2", name="r2", bufs=2)
        with nc.allow_low_precision(reason="softmax denominators are O(1); fp32r rounding is fine"):
            nc.vector.reciprocal(out=r2, in_=s2)

        def apply(half, r2=r2, pg=pg):
            pb = pproj.tile([P, 512], F32, tag="pp", name="ppb")
            for a in range(4 * half, 4 * half + 4):
                nc.tensor.matmul(
                    pb[:, (a % 4) * P:(a % 4 + 1) * P], sel[a], r2,
                    start=True, stop=True, skip_group_check=True,
                )
            nc.vector.tensor_mul(
                zt[pg][:, half * 512:half * 512 + 512],
                zt[pg][:, half * 512:half * 512 + 512],
                pb,
            )

        for half in (0, 1):
            filler.append((None, False, lambda half=half: apply(half)))

    def pop_filler(allow_late):
        for idx, (pg, early_ok, emit) in enumerate(filler):
            if early_ok or allow_late:
                filler.pop(idx)
                emit()
                return

    def drain_pair(g):
        # everything pair g depends on must be emitted before its attention
        mine = [u for u in filler if u[0] == g]
        filler[:] = [u for u in filler if u[0] != g]
        for u in mine:
            u[2]()

    # ---- phase 2: v projection -> va [m, 16*65] with ones columns,
    #      with q/k projections for pairs 0 and 1 riding the filler queue ----
    push_proj_pair(0)
    push_proj_pair(1)
    with tc.tile_pool(name="pv", bufs=2, space="PSUM") as pv:
        for t in range(ST):
            pt = pv.tile([P, VTOT], F32, tag="pv", name="pvt")
            for c in range(DCH):
                for n0 in range(0, VTOT, 512):
                    nw = min(512, VTOT - n0)
                    nc.tensor.matmul(
                        pt[:, n0:n0 + nw],
                        nrpT[c][:, t * P:(t + 1) * P],
                        vw[c][:, n0:n0 + nw],
                        start=(c == 0), stop=(c == DCH - 1),
                        skip_group_check=True,
                    )
            if t >= 1:
                pop_filler(allow_late=True)
                pop_filler(allow_late=True)
            # vb_bc has the per-(h,dh) bias, with 1.0 in each ones-column slot;
            # matmul wrote 0 there (vst ones-columns are zero), so add gives 1.0
            nc.vector.tensor_add(va[t], pt, vb_bc)

    nrp_t_pool.release()

    # ---- phase 3: attention; later pairs' q/k projections and earlier
    #      pairs' softmax normalization ride the PE filler queue ----
    with tc.tile_pool(name="attn", bufs=3) as apool, \
         tc.tile_pool(name="ps_s", bufs=2, space="PSUM") as spool, \
         tc.tile_pool(name="ps_z", bufs=1, space="PSUM") as zpool:
        for h in range(N_HEADS):
            g, off = h // 2, (h % 2) * D_HEAD
            if h % 2 == 0:
                drain_pair(g)
                if g >= 1:
                    push_norm_pair(g - 1)
                if g + 2 < PAIRS:
                    push_proj_pair(g + 2)
            pz = zpool.tile([VW, SEQ], F32, tag="pz", name="pz")

            def av_mms(i, ae):
                q0 = i * P
                for n0, nw in _bank_splits(q0):
                    nc.tensor.matmul(
                        pz[:, n0:n0 + nw],
                        va[i][:, h * VW:(h + 1) * VW],
                        ae[:, n0:n0 + nw],
                        start=(i == 0), stop=(i == ST - 1),
                        skip_group_check=True,
                    )

            pend = None
            for i in range(ST):
                q0 = i * P
                ps = spool.tile([P, SEQ], F32, tag="ps", name="ps")
                ae = apool.tile([P, SEQ], BF16, tag="ae", name="ae")
                for n0, nw in _bank_splits(q0):
                    nc.tensor.matmul(
                        ps[:, n0:n0 + nw],
                        kt[g][off:off + D_HEAD, q0:q0 + P],
                        qt[g][off:off + D_HEAD, n0:n0 + nw],
                        start=True, stop=(n0 != q0),
                        skip_group_check=True,
                    )
                # causal diag mask: accumulate I.T @ M0
                nc.tensor.matmul(
                    ps[:, q0:q0 + P],
                    ident_bf, mask_bf,
                    start=False, stop=True,
                    skip_group_check=True,
                )
                if i % 2 == 0:
                    pop_filler(allow_late=(i >= 4))
                nc.scalar.activation(
                    out=ae[:, q0:SEQ], in_=ps[:, q0:SEQ],
                    func=AF.Exp, scale=float(SCALE),
                )
                # attn@v delayed one chunk so exp latency hides behind PE work
                if pend is not None:
                    av_mms(*pend)
                pend = (i, ae)
            av_mms(*pend)
            # denominator row first (feeds the normalization chain), then
            # stash unnormalized zT; both free the PSUM slot
            srow = rpool.tile([1, SEQ], F32, tag="srow", name="srow", bufs=2)
            nc.vector.tensor_copy(srow, pz[D_HEAD:VW, :])
            nc.sync.dma_start(out=sums_dram[h:h + 1, :], in_=srow)
            nc.vector.tensor_copy(zt[g][off:off + D_HEAD, :], pz[0:D_HEAD, :])

        push_norm_pair(PAIRS - 1)

    # ---- phase 4: output projection out[s, d] = zt.T @ O + ob.
    # The first two seq-tiles' k=0..6 accumulations overlap the final
    # pair's normalization chain; their k=7 matmuls are issued after it.
    with tc.tile_pool(name="outsb", bufs=3) as outsb, \
         tc.tile_pool(name="po", bufs=2, space="PSUM") as po:
        def out_mms(pt, t, k):
            for n0 in range(0, D_MODEL, 512):
                nc.tensor.matmul(
                    pt[:, n0:n0 + 512],
                    zt[k][:, t * P:(t + 1) * P],
                    owt[k][:, n0:n0 + 512],
                    start=(k == 0), stop=(k == DCH - 1),
                    skip_group_check=True,
                )

        def out_drain(pt, t):
            ot = outsb.tile([P, D_MODEL], F32, tag="ot", name="ot")
            nc.vector.tensor_add(ot, pt, ob_bc)
            nc.sync.dma_start(out=out[t * P:(t + 1) * P, :], in_=ot)

        head_pts = []
        for t in (0, 1):
            pt = po.tile([P, D_MODEL], F32, tag="po", name="pot")
            head_pts.append(pt)
            for k in range(DCH - 1):
                out_mms(pt, t, k)
        while filler:
            pop_filler(allow_late=True)
        for t in (0, 1):
            out_mms(head_pts[t], t, DCH - 1)
            out_drain(head_pts[t], t)
        for t in range(2, ST):
            pt = po.tile([P, D_MODEL], F32, tag="po", name="pot")
            for k in range(DCH):
                out_mms(pt, t, k)
            out_drain(pt, t)

    pproj.release()
    for pool in (rpool, zt_pool, va_pool, kt_pool, qt_pool, enc_t_pool, wpool, smalls):
        pool.release()


def _get_program():
    if "nc" not in _CACHE:
        _CACHE["nc"] = _build_program()
    return _CACHE["nc"]


def _pack_weights(Qs, Qbs, Ks, Kbs, Vs, Vbs, O, Ob):
    f = np.float32
    qst = np.ascontiguousarray(np.transpose(np.asarray(Qs, f), (1, 0, 2)).reshape(D_MODEL, D_MODEL)).astype(NPBF16)
    kst = np.ascontiguousarray(np.transpose(np.asarray(Ks, f), (1, 0, 2)).reshape(D_MODEL, D_MODEL)).astype(NPBF16)
    vst = np.zeros((D_MODEL, VTOT), f)
    vb = np.zeros((VTOT,), f)
    Vs = np.asarray(Vs, f)
    Vbs = np.asarray(Vbs, f)
    for h in range(N_HEADS):
        vst[:, h * VW:h * VW + D_HEAD] = Vs[h]
        vb[h * VW:h * VW + D_HEAD] = Vbs[h]
        vb[h * VW + D_HEAD] = 1.0
    vst = vst.astype(NPBF16)
    ow = np.ascontiguousarray(np.asarray(O, f).reshape(D_MODEL, D_MODEL)).astype(NPBF16)
    qbf = np.asarray(Qbs, f).reshape(D_MODEL)
    kbf = np.asarray(Kbs, f).reshape(D_MODEL)
    obf = np.asarray(Ob, f).reshape(D_MODEL)
    # row-replicated bias pack: [0:8] qb by (pair, partition), [8:16] kb,
    # [16:1056] vb broadcast, [1056:2080] ob broadcast
    biases = np.empty((P, 16 + VTOT + D_MODEL), f)
    biases[:, 0:PAIRS] = qbf.reshape(PAIRS, P).T
    biases[:, PAIRS:2 * PAIRS] = kbf.reshape(PAIRS, P).T
    biases[:, 16:16 + VTOT] = vb[None, :]
    biases[:, 16 + VTOT:] = obf[None, :]
    return qst, kst, vst, ow, np.ascontiguousarray(biases)


def kernel(normalized_resid_pre, encoder_output, Qs, Qbs, Ks, Kbs, Vs, Vbs, O, Ob,
           _trace=False, _trace_kwargs=None):
    nc = _get_program()
    qst, kst, vst, ow, biases = _pack_weights(Qs, Qbs, Ks, Kbs, Vs, Vbs, O, Ob)
    enc = np.asarray(encoder_output, np.float32).astype(NPBF16)
    nrp = np.asarray(normalized_resid_pre, np.float32).astype(NPBF16)
    in_maps = []
    for b in range(BATCH):
        in_maps.append({
            "enc": np.ascontiguousarray(enc[b]),
            "nrp": np.ascontiguousarray(nrp[b]),
            "qst": qst, "kst": kst, "vst": vst, "ow": ow,
            "biases": biases,
        })
    res = run_bass_kernel_spmd(
        nc, in_maps, list(range(BATCH)),
        trace=_trace, **(_trace_kwargs or {}),
    )
    out = np.stack([res.results[b]["out"] for b in range(BATCH)], axis=0)
    if _trace:
        _CACHE["last_results"] = res
    return out


# revision 20
# speedup vs baseline: 1.6313x; 1.1073x over previous
"""DecoderAttention Bass/Tile kernel for TRN2, batch-parallel over 8 NeuronCores.

Each core handles one batch element:
  q = enc @ Qs + Qbs ; k = enc @ Ks + Kbs ; v = nrp @ Vs + Vbs   (per head)
  scores = q k^T / sqrt(64), causal mask (-1e5), softmax
  out = (attn @ v) @ O + Ob

Layout/throughput strategy (all matmuls in bf16 at 1 col/cycle, f32 PSUM):
  - all big DRAM inputs pre-cast to bf16 host-side (halves DMA, enables
    bf16 PE transposes and 2x matmul rate vs fp32r; measured end-to-end
    max rel err ~4e-3 vs the 2e-2 gate)
  - enc/nrp transposed on-device (PE transpose, bf16 PSUM) to [d, s];
    nrp first so the v projection can start as early as possible; each
    seq-tile load is split across the sync and vector DMA queues (one
    queue sustains only ~150 GB/s on 2KB-per-partition patterns), and
    bias/broadcast DMAs ride the gpsimd queue so they never head-of-line
    block the transpose feed
  - a burst of dummy matmuls at t=0 holds the PE busy through one full
    HAM activity window, lifting the clock gate to 2.4 GHz early
  - weights pre-packed host-side to [d, (h dh)]; Vs padded to [d, 16*65]
    with a ones column per head so attn@v also produces softmax row sums
  - scoresT [m, q] per head so exp output feeds attn@v without transposing
  - causal diagonal blocks masked by accumulating I.T @ M0 (bf16) in PSUM
  - exp folds the 1/sqrt(d_head) scale; no max subtraction (scores are O(1),
    masked entries become exactly 0)
  - the HAM clock gate re-throttles whenever PE duty sags inside a ~3.4us
    window, so independent PE work rides a fine-grained filler queue
    threaded through the v-projection and attention loops: q/k projection
    half-groups (4 matmuls) for later pairs, and the softmax-normalization
    broadcast matmuls of earlier pairs (marked late so the PE never blocks
    on their reciprocal chain); units are pair-tagged and force-drained
    before the pair that needs them
  - softmax division deferred: per-pair reciprocal over sums reshaped to
    [16, 128] (partition-parallel reciprocal), broadcast back across
    partitions with 8 one-hot selector matmuls, applied to zt while later
    pairs run; the final pair's normalization overlaps the first output-
    projection accumulations (their k=7 contribution is issued last)
"""

import numpy as np
import ml_dtypes

import concourse.bass as bass
import concourse.mybir as mybir
import concourse.tile as tile
from concourse import bacc
from concourse.bass_utils import run_bass_kernel_spmd
from concourse.masks import make_identity

N_HEADS, D_MODEL, D_HEAD = 16, 1024, 64
BATCH, SEQ = 8, 1024
P = 128
DCH = D_MODEL // P       # 8 contraction chunks
ST = SEQ // P            # 8 seq tiles
PAIRS = N_HEADS // 2     # 8 head pairs
VW = 65                  # v width per head incl. ones column
VTOT = N_HEADS * VW      # 1040
IGNORE = -100000.0
SCALE = 1.0 / np.sqrt(np.float32(D_HEAD))

F32 = mybir.dt.float32
F32R = mybir.dt.float32r
BF16 = mybir.dt.bfloat16
AF = mybir.ActivationFunctionType
NPBF16 = ml_dtypes.bfloat16

_CACHE = {}


def _bank_splits(q0):
    # PSUM-bank-aligned (n0, nw) column splits covering [q0, SEQ)
    if q0 < 512:
        return [(q0, 512 - q0), (512, 512)]
    return [(q0, SEQ - q0)]


def _bcast_row_ap(src, n):
    # DMA access pattern replicating a [n]-element DRAM row to 128 partitions
    return bass.AP(tensor=src.tensor, offset=src.offset, ap=[[0, P], [1, n]])


def _build_program():
    nc = bacc.Bacc("TRN2", target_bir_lowering=False, debug=False, num_devices=8)

    enc = nc.dram_tensor("enc", [SEQ, D_MODEL], BF16, kind="ExternalInput").ap()
    nrp = nc.dram_tensor("nrp", [SEQ, D_MODEL], BF16, kind="ExternalInput").ap()
    qst = nc.dram_tensor("qst", [D_MODEL, D_MODEL], BF16, kind="ExternalInput").ap()
    kst = nc.dram_tensor("kst", [D_MODEL, D_MODEL], BF16, kind="ExternalInput").ap()
    vst = nc.dram_tensor("vst", [D_MODEL, VTOT], BF16, kind="ExternalInput").ap()
    ow = nc.dram_tensor("ow", [D_MODEL, D_MODEL], BF16, kind="ExternalInput").ap()
    # all biases host-packed into one row-replicated [128, 2080] tensor:
    # cols [0:8] qb by pair, [8:16] kb by pair, [16:1056] vb, [1056:2080] ob
    biases = nc.dram_tensor("biases", [P, 16 + VTOT + D_MODEL], F32, kind="ExternalInput").ap()
    out = nc.dram_tensor("out", [SEQ, D_MODEL], BF16, kind="ExternalOutput").ap()
    sums_dram = nc.dram_tensor("sums_scratch", [N_HEADS, SEQ], F32).ap()

    with tile.TileContext(nc) as tc:
        _kernel(tc, out, enc, nrp, qst, kst, vst, ow, biases,
                sums_dram=sums_dram)
    nc.compile()
    return nc


def _kernel(tc, out, enc, nrp, qst, kst, vst, ow, biases, sums_dram=None):
    nc = tc.nc

    smalls = tc.alloc_tile_pool(name="smalls", bufs=1)
    ident_bf = smalls.tile([P, P], BF16, tag="ident_bf", name="ident_bf")
    make_identity(nc, ident_bf)
    # M0[m, q] = IGNORE where m > q else 0 (strict causal mask, diag block)
    mask_bf = smalls.tile([P, P], BF16, tag="mask_bf", name="mask_bf")
    nc.gpsimd.memset(mask_bf, 0.0)
    nc.gpsimd.affine_select(
        out=mask_bf, in_=mask_bf,
        compare_op=mybir.AluOpType.is_ge,
        fill=IGNORE, base=0,
        pattern=[[1, P]], channel_multiplier=-1,
    )
    # sel[a][j, p] = 1 where j == (p // 64) * 8 + a: broadcasts the [16, 128]
    # reciprocal layout (row = (head, seg), col = q%128) to [128, q-seg a]
    sel = []
    self_f = smalls.tile([N_HEADS, P], F32, tag="self", name="self")
    for a in range(DCH):
        nc.gpsimd.memset(self_f, 0.0)
        nc.gpsimd.affine_select(
            out=self_f.rearrange("j (h c) -> j h c", h=2),
            in_=self_f.rearrange("j (h c) -> j h c", h=2),
            compare_op=mybir.AluOpType.not_equal,
            fill=1.0, base=-a,
            pattern=[[-8, 2], [0, D_HEAD]], channel_multiplier=1,
        )
        s_r = smalls.tile([N_HEADS, P], F32R, tag=f"sel{a}", name=f"sel{a}")
        nc.vector.tensor_copy(s_r, self_f)
        sel.append(s_r)
    bias_sb = smalls.tile([P, 16 + VTOT + D_MODEL], F32, tag="bias_sb", name="bias_sb")
    qb_col = bias_sb[:, 0:PAIRS]
    kb_col = bias_sb[:, PAIRS:2 * PAIRS]
    vb_bc = bias_sb[:, 16:16 + VTOT]
    ob_bc = bias_sb[:, 16 + VTOT:16 + VTOT + D_MODEL]

    # persistent weight tiles (bf16), prefetched on the scalar DMA queue in
    # consumption order (v projection first, output projection last)
    wpool = tc.alloc_tile_pool(name="weights", bufs=1, side="right")
    vw = [wpool.tile([P, VTOT], BF16, tag=f"vw{c}", name=f"vw{c}") for c in range(DCH)]
    qw = [wpool.tile([P, D_MODEL], BF16, tag=f"qw{c}", name=f"qw{c}") for c in range(DCH)]
    kw = [wpool.tile([P, D_MODEL], BF16, tag=f"kw{c}", name=f"kw{c}") for c in range(DCH)]
    owt = [wpool.tile([P, D_MODEL], BF16, tag=f"owt{c}", name=f"owt{c}") for c in range(DCH)]
    for c in range(DCH):
        nc.scalar.dma_start(out=vw[c], in_=vst[c * P:(c + 1) * P, :])
    for c in range(DCH):
        nc.scalar.dma_start(out=qw[c], in_=qst[c * P:(c + 1) * P, :])
    for c in range(DCH):
        nc.scalar.dma_start(out=kw[c], in_=kst[c * P:(c + 1) * P, :])
    for c in range(DCH):
        nc.scalar.dma_start(out=owt[c], in_=ow[c * P:(c + 1) * P, :])

    enc_t_pool = tc.alloc_tile_pool(name="encT", bufs=1, side="right")
    nrp_t_pool = tc.alloc_tile_pool(name="nrpT", bufs=1, side="right")
    encT = [enc_t_pool.tile([P, SEQ], BF16, tag=f"encT{c}", name=f"encT{c}") for c in range(DCH)]
    nrpT = [nrp_t_pool.tile([P, SEQ], BF16, tag=f"nrpT{c}", name=f"nrpT{c}") for c in range(DCH)]

    # ---- phase 1: dummy-matmul burst to lift the HAM clock gate, then
    #      transpose nrp into [d, s] (bf16). enc is transposed later,
    #      interleaved into the v projection so its DMA latency hides
    #      behind matmul work. Chunk-grouped 3/3/2 so the transpose PSUM
    #      tiles fit alongside the v-projection accumulator. ----
    with tc.tile_pool(name="warmps", bufs=1, space="PSUM") as wps:
        warm = wps.tile([P, P], F32, tag="warm", name="warm")
        for _ in range(32):
            nc.tensor.matmul(warm, ident_bf, ident_bf, start=True, stop=True,
                             skip_group_check=True)
    CG = ((0, 1, 2), (3, 4, 5), (6, 7))
    trin = tc.alloc_tile_pool(name="trin", bufs=4, side="right")
    pproj = tc.alloc_tile_pool(name="pproj", bufs=2, space="PSUM")
    trps = tc.alloc_tile_pool(name="trps", bufs=1, space="PSUM")

    def transpose_block(src, dst, tq):
        # two seq-tiles of src -> dst[c][:, tq*P:(tq+2)*P]
        s_ins = []
        for t in range(tq, tq + 2):
            s_in = trin.tile([P, D_MODEL], BF16, tag="s_in", name="s_in")
            nc.sync.dma_start(out=s_in, in_=src[t * P:(t + 1) * P, :])
            s_ins.append(s_in)
        for cg in CG:
            ptiles = {c: trps.tile([P, 2 * P], BF16, tag=f"tr{ci}", name=f"tr{ci}")
                      for ci, c in enumerate(cg)}
            for ti, s_in in enumerate(s_ins):
                for c in cg:
                    nc.tensor.transpose(
                        ptiles[c][:, ti * P:(ti + 1) * P],
                        s_in[:, c * P:(c + 1) * P],
                        ident_bf,
                    )
            for c in cg:
                nc.any.tensor_copy(dst[c][:, tq * P:(tq + 2) * P], ptiles[c])

    for tq in range(0, ST, 2):
        transpose_block(nrp, nrpT, tq)
    nc.sync.dma_start(out=bias_sb, in_=biases)

    qt_pool = tc.alloc_tile_pool(name="qt", bufs=1)
    kt_pool = tc.alloc_tile_pool(name="kt", bufs=1)
    qt = [qt_pool.tile([P, SEQ], BF16, tag=f"qt{g}", name=f"qt{g}") for g in range(PAIRS)]
    kt = [kt_pool.tile([P, SEQ], BF16, tag=f"kt{g}", name=f"kt{g}") for g in range(PAIRS)]
    va_pool = tc.alloc_tile_pool(name="va", bufs=1)
    va = [va_pool.tile([P, VTOT], BF16, tag=f"va{t}", name=f"va{t}") for t in range(ST)]
    zt_pool = tc.alloc_tile_pool(name="zt", bufs=1)
    zt = [zt_pool.tile([P, SEQ], BF16, tag=f"zt{k}", name=f"zt{k}") for k in range(DCH)]

    rpool = tc.alloc_tile_pool(name="rnorm", bufs=1)

    # ---- PE filler queue: (pair, early_ok, emit). Early slots sit right
    # behind fresh attention matmuls, so only latency-free work goes there.
    # Units are pair-tagged so everything pair g needs is force-drained
    # before its attention begins. ----
    filler = []
    proj_state = {}

    def proj_half_mms(g, w, bcol, dst, n0, chalf):
        key = (g, id(w), n0)
        if chalf == 0:
            proj_state[key] = pproj.tile([P, 512], F32, tag="pp", name="pp")
        pp = proj_state[key]
        for c in range(4 * chalf, 4 * chalf + 4):
            nc.tensor.matmul(
                pp,
                w[c][:, g * P:(g + 1) * P],
                encT[c][:, n0:n0 + 512],
                start=(c == 0), stop=(c == DCH - 1),
                skip_group_check=True,
            )
        if chalf == 1:
            del proj_state[key]
            nc.vector.tensor_scalar_add(
                out=dst[g][:, n0:n0 + 512], in0=pp, scalar1=bcol[:, g:g + 1],
            )

    def push_proj_pair(g):
        for w, bcol, dst in ((qw, qb_col, qt), (kw, kb_col, kt)):
            for n0 in (0, 512):
                for chalf in (0, 1):
                    filler.append((g, True,
                                   lambda g=g, w=w, bcol=bcol, dst=dst, n0=n0, chalf=chalf:
                                   proj_half_mms(g, w, bcol, dst, n0, chalf)))

    def push_norm_pair(pg):
        # reciprocal of softmax sums for pair pg: gather the two sum rows as
        # [16, 128] so the FD-bound reciprocal runs across partitions, then
        # broadcast to [128, SEQ] via the one-hot selector matmuls.
        s2 = rpool.tile([N_HEADS, P], F32, tag="s2", name="s2", bufs=2)
        nc.sync.dma_start(
            out=s2,
            in_=sums_dram[2 * pg:2 * pg + 2, :].rearrange("h (a c) -> (h a) c", c=P),
        )
        r2 = rpool.tile([N_HEADS, P], F32R, tag="r2", name="r2", bufs=2)
        with nc.allow_low_precision(reason="softmax denominators are O(1); fp32r rounding is fine"):
            nc.vector.reciprocal(out=r2, in_=s2)

        def apply(half, r2=r2, pg=pg):
            pb = pproj.tile([P, 512], F32, tag="pp", name="ppb")
            for a in range(4 * half, 4 * half + 4):
                nc.tensor.matmul(
                    pb[:, (a % 4) * P:(a % 4 + 1) * P], sel[a], r2,
                    start=True, stop=True, skip_group_check=True,
                )
            nc.vector.tensor_mul(
                zt[pg][:, half * 512:half * 512 + 512],
                zt[pg][:, half * 512:half * 512 + 512],
                pb,
            )

        for half in (0, 1):
            filler.append((None, False, lambda half=half: apply(half)))

    def pop_filler(allow_late):
        for idx, (pg, early_ok, emit) in enumerate(filler):
            if early_ok or allow_late:
                filler.pop(idx)
                emit()
                return

    def drain_pair(g):
        # everything pair g depends on must be emitted before its attention
        mine = [u for u in filler if u[0] == g]
        filler[:] = [u for u in filler if u[0] != g]
        for u in mine:
            u[2]()

    # ---- phase 2: v projection -> va [m, 16*65] with ones columns.
    # enc transpose blocks and the q/k projections for pairs 0 and 1 are
    # interleaved so the PE never idles on the enc DMA stream. The n0=0
    # projection halves only need enc seq-tiles 0-3, which are transposed
    # by t=3; n0=512 halves are force-drained at attention start. ----
    for g01 in (0, 1):
        for w, bcol, dst in ((qw, qb_col, qt), (kw, kb_col, kt)):
            for chalf in (0, 1):
                filler.append((g01, True,
                               lambda g=g01, w=w, bcol=bcol, dst=dst, chalf=chalf:
                               proj_half_mms(g, w, bcol, dst, 0, chalf)))
    for g01 in (0, 1):
        for w, bcol, dst in ((qw, qb_col, qt), (kw, kb_col, kt)):
            for chalf in (0, 1):
                filler.append((g01, True,
                               lambda g=g01, w=w, bcol=bcol, dst=dst, chalf=chalf:
                               proj_half_mms(g, w, bcol, dst, 512, chalf)))
    with tc.tile_pool(name="pv", bufs=1, space="PSUM") as pv:
        for t in range(ST):
            pt = pv.tile([P, VTOT], F32, tag="pv", name="pvt")
            for c in range(DCH):
                for n0 in range(0, VTOT, 512):
                    nw = min(512, VTOT - n0)
                    nc.tensor.matmul(
                        pt[:, n0:n0 + nw],
                        nrpT[c][:, t * P:(t + 1) * P],
                        vw[c][:, n0:n0 + nw],
                        start=(c == 0), stop=(c == DCH - 1),
                        skip_group_check=True,
                    )
            if t < 4:
                transpose_block(enc, encT, 2 * t)
            else:
                pop_filler(allow_late=True)
                pop_filler(allow_late=True)
            # vb_bc has the per-(h,dh) bias, with 1.0 in each ones-column slot;
            # matmul wrote 0 there (vst ones-columns are zero), so add gives 1.0
            nc.vector.tensor_add(va[t], pt, vb_bc)

    trps.release()
    trin.release()
    nrp_t_pool.release()

    # ---- phase 3: attention; later pairs' q/k projections and earlier
    #      pairs' softmax normalization ride the PE filler queue ----
    with tc.tile_pool(name="attn", bufs=3) as apool, \
         tc.tile_pool(name="ps_s", bufs=2, space="PSUM") as spool, \
         tc.tile_pool(name="ps_z", bufs=1, space="PSUM") as zpool:
        for h in range(N_HEADS):
            g, off = h // 2, (h % 2) * D_HEAD
            if h % 2 == 0:
                drain_pair(g)
                if g >= 1:
                    push_norm_pair(g - 1)
                if g + 2 < PAIRS:
                    push_proj_pair(g + 2)
            pz = zpool.tile([VW, SEQ], F32, tag="pz", name="pz")

            def av_mms(i, ae):
                q0 = i * P
                for n0, nw in _bank_splits(q0):
                    nc.tensor.matmul(
                        pz[:, n0:n0 + nw],
                        va[i][:, h * VW:(h + 1) * VW],
                        ae[:, n0:n0 + nw],
                        start=(i == 0), stop=(i == ST - 1),
                        skip_group_check=True,
                    )

            pend = None
            for i in range(ST):
                q0 = i * P
                ps = spool.tile([P, SEQ], F32, tag="ps", name="ps")
                ae = apool.tile([P, SEQ], BF16, tag="ae", name="ae")
                for n0, nw in _bank_splits(q0):
                    nc.tensor.matmul(
                        ps[:, n0:n0 + nw],
                        kt[g][off:off + D_HEAD, q0:q0 + P],
                        qt[g][off:off + D_HEAD, n0:n0 + nw],
                        start=True, stop=(n0 != q0),
                        skip_group_check=True,
                    )
                # causal diag mask: accumulate I.T @ M0
                nc.tensor.matmul(
                    ps[:, q0:q0 + P],
                    ident_bf, mask_bf,
                    start=False, stop=True,
                    skip_group_check=True,
                )
                if i % 2 == 0:
                    pop_filler(allow_late=(i >= 4))
                nc.scalar.activation(
                    out=ae[:, q0:SEQ], in_=ps[:, q0:SEQ],
                    func=AF.Exp, scale=float(SCALE),
                )
                # attn@v delayed one chunk so exp latency hides behind PE work
                if pend is not None:
                    av_mms(*pend)
                pend = (i, ae)
            av_mms(*pend)
            # denominator row first (feeds the normalization chain), then
            # stash unnormalized zT; both free the PSUM slot
            srow = rpool.tile([1, SEQ], F32, tag="srow", name="srow", bufs=2)
            nc.vector.tensor_copy(srow, pz[D_HEAD:VW, :])
            nc.sync.dma_start(out=sums_dram[h:h + 1, :], in_=srow)
            nc.vector.tensor_copy(zt[g][off:off + D_HEAD, :], pz[0:D_HEAD, :])

        push_norm_pair(PAIRS - 1)

    # ---- phase 4: output projection out[s, d] = zt.T @ O + ob.
    # The first two seq-tiles' k=0..6 accumulations overlap the final
    # pair's normalization chain; their k=7 matmuls are issued after it.
    with tc.tile_pool(name="outsb", bufs=3) as outsb, \
         tc.tile_pool(name="po", bufs=2, space="PSUM") as po:
        def out_mms(pt, t, k):
            for n0 in range(0, D_MODEL, 512):
                nc.tensor.matmul(
                    pt[:, n0:n0 + 512],
                    zt[k][:, t * P:(t + 1) * P],
                    owt[k][:, n0:n0 + 512],
                    start=(k == 0), stop=(k == DCH - 1),
                    skip_group_check=True,
                )

        def out_drain(pt, t):
            # bf16 output (upcast host-side) halves the writeback traffic,
            # striped over three DMA queues so the tail drains fast
            ot = outsb.tile([P, D_MODEL], BF16, tag="ot", name="ot")
            nc.vector.tensor_add(ot, pt, ob_bc)
            eng = (nc.sync, nc.scalar, nc.gpsimd)[t % 3]
            eng.dma_start(out=out[t * P:(t + 1) * P, :], in_=ot)

        head_pts = []
        for t in (0, 1):
            pt = po.tile([P, D_MODEL], F32, tag="po", name="pot")
            head_pts.append(pt)
            for k in range(DCH - 1):
                out_mms(pt, t, k)
        while filler:
            pop_filler(allow_late=True)
        for t in (0, 1):
            out_mms(head_pts[t], t, DCH - 1)
            out_drain(head_pts[t], t)
        for t in range(2, ST):
            pt = po.tile([P, D_MODEL], F32, tag="po", name="pot")
            for k in range(DCH):
                out_mms(pt, t, k)
            out_drain(pt, t)

    pproj.release()
    for pool in (rpool, zt_pool, va_pool, kt_pool, qt_pool, enc_t_pool, wpool, smalls):
        pool.release()


def _get_program():
    if "nc" not in _CACHE:
        _CACHE["nc"] = _build_program()
    return _CACHE["nc"]


def _pack_weights(Qs, Qbs, Ks, Kbs, Vs, Vbs, O, Ob):
    f = np.float32
    qst = np.ascontiguousarray(np.transpose(np.asarray(Qs, f), (1, 0, 2)).reshape(D_MODEL, D_MODEL)).astype(NPBF16)
    kst = np.ascontiguousarray(np.transpose(np.asarray(Ks, f), (1, 0, 2)).reshape(D_MODEL, D_MODEL)).astype(NPBF16)
    vst = np.zeros((D_MODEL, VTOT), f)
    vb = np.zeros((VTOT,), f)
    Vs = np.asarray(Vs, f)
    Vbs = np.asarray(Vbs, f)
    for h in range(N_HEADS):
        vst[:, h * VW:h * VW + D_HEAD] = Vs[h]
        vb[h * VW:h * VW + D_HEAD] = Vbs[h]
        vb[h * VW + D_HEAD] = 1.0
    vst = vst.astype(NPBF16)
    ow = np.ascontiguousarray(np.asarray(O, f).reshape(D_MODEL, D_MODEL)).astype(NPBF16)
    qbf = np.asarray(Qbs, f).reshape(D_MODEL)
    kbf = np.asarray(Kbs, f).reshape(D_MODEL)
    obf = np.asarray(Ob, f).reshape(D_MODEL)
    # row-replicated bias pack: [0:8] qb by (pair, partition), [8:16] kb,
    # [16:1056] vb broadcast, [1056:2080] ob broadcast
    biases = np.empty((P, 16 + VTOT + D_MODEL), f)
    biases[:, 0:PAIRS] = qbf.reshape(PAIRS, P).T
    biases[:, PAIRS:2 * PAIRS] = kbf.reshape(PAIRS, P).T
    biases[:, 16:16 + VTOT] = vb[None, :]
    biases[:, 16 + VTOT:] = obf[None, :]
    return qst, kst, vst, ow, np.ascontiguousarray(biases)


def kernel(normalized_resid_pre, encoder_output, Qs, Qbs, Ks, Kbs, Vs, Vbs, O, Ob,
           _trace=False, _trace_kwargs=None):
    nc = _get_program()
    qst, kst, vst, ow, biases = _pack_weights(Qs, Qbs, Ks, Kbs, Vs, Vbs, O, Ob)
    enc = np.asarray(encoder_output, np.float32).astype(NPBF16)
    nrp = np.asarray(normalized_resid_pre, np.float32).astype(NPBF16)
    in_maps = []
    for b in range(BATCH):
        in_maps.append({
            "enc": np.ascontiguousarray(enc[b]),
            "nrp": np.ascontiguousarray(nrp[b]),
            "qst": qst, "kst": kst, "vst": vst, "ow": ow,
            "biases": biases,
        })
    res = run_bass_kernel_spmd(
        nc, in_maps, list(range(BATCH)),
        trace=_trace, **(_trace_kwargs or {}),
    )
    out = np.stack([np.asarray(res.results[b]["out"]).astype(np.float32) for b in range(BATCH)], axis=0)
    if _trace:
        _CACHE["last_results"] = res
    return out


# revision 24
# speedup vs baseline: 1.7583x; 1.0778x over previous
"""DecoderAttention Bass/Tile kernel for TRN2, batch-parallel over 8 NeuronCores.

Each core handles one batch element:
  q = enc @ Qs + Qbs ; k = enc @ Ks + Kbs ; v = nrp @ Vs + Vbs   (per head)
  scores = q k^T / sqrt(64), causal mask (-1e5), softmax
  out = (attn @ v) @ O + Ob

Layout/throughput strategy (all matmuls in bf16 at 1 col/cycle, f32 PSUM):
  - all big DRAM inputs pre-cast to bf16 host-side (halves DMA, enables
    bf16 PE transposes and 2x matmul rate vs fp32r; measured end-to-end
    max rel err ~4e-3 vs the 2e-2 gate)
  - enc/nrp transposed on-device (PE transpose, bf16 PSUM) to [d, s];
    nrp first so the v projection can start as early as possible; each
    seq-tile load is split across the sync and vector DMA queues (one
    queue sustains only ~150 GB/s on 2KB-per-partition patterns), and
    bias/broadcast DMAs ride the gpsimd queue so they never head-of-line
    block the transpose feed
  - a burst of dummy matmuls at t=0 holds the PE busy through one full
    HAM activity window, lifting the clock gate to 2.4 GHz early
  - weights pre-packed host-side to [d, (h dh)]; Vs padded to [d, 16*65]
    with a ones column per head so attn@v also produces softmax row sums
  - scoresT [m, q] per head so exp output feeds attn@v without transposing
  - causal diagonal blocks masked by accumulating I.T @ M0 (bf16) in PSUM
  - exp folds the 1/sqrt(d_head) scale; no max subtraction (scores are O(1),
    masked entries become exactly 0)
  - the HAM clock gate re-throttles whenever PE duty sags inside a ~3.4us
    window, so independent PE work rides a fine-grained filler queue
    threaded through the v-projection and attention loops: q/k projection
    half-groups (4 matmuls) for later pairs, and the softmax-normalization
    broadcast matmuls of earlier pairs (marked late so the PE never blocks
    on their reciprocal chain); units are pair-tagged and force-drained
    before the pair that needs them
  - softmax division deferred: per-pair reciprocal over sums reshaped to
    [16, 128] (partition-parallel reciprocal), broadcast back across
    partitions with 8 one-hot selector matmuls, applied to zt while later
    pairs run; the final pair's normalization overlaps the first output-
    projection accumulations (their k=7 contribution is issued last)
"""

import numpy as np
import ml_dtypes

import concourse.bass as bass
import concourse.mybir as mybir
import concourse.tile as tile
from concourse import bacc
from concourse.bass_utils import run_bass_kernel_spmd
from concourse.masks import make_identity

N_HEADS, D_MODEL, D_HEAD = 16, 1024, 64
BATCH, SEQ = 8, 1024
P = 128
DCH = D_MODEL // P       # 8 contraction chunks
ST = SEQ // P            # 8 seq tiles
PAIRS = N_HEADS // 2     # 8 head pairs
VW = 65                  # v width per head incl. ones column
VTOT = N_HEADS * VW      # 1040
IGNORE = -100000.0
SCALE = 1.0 / np.sqrt(np.float32(D_HEAD))

F32 = mybir.dt.float32
F32R = mybir.dt.float32r
BF16 = mybir.dt.bfloat16
AF = mybir.ActivationFunctionType
NPBF16 = ml_dtypes.bfloat16

_CACHE = {}


def _bank_splits(q0):
    # PSUM-bank-aligned (n0, nw) column splits covering [q0, SEQ)
    if q0 < 512:
        return [(q0, 512 - q0), (512, 512)]
    return [(q0, SEQ - q0)]


def _bcast_row_ap(src, n):
    # DMA access pattern replicating a [n]-element DRAM row to 128 partitions
    return bass.AP(tensor=src.tensor, offset=src.offset, ap=[[0, P], [1, n]])


def _build_program():
    nc = bacc.Bacc("TRN2", target_bir_lowering=False, debug=False, num_devices=8)

    enc = nc.dram_tensor("enc", [SEQ, D_MODEL], BF16, kind="ExternalInput").ap()
    nrp = nc.dram_tensor("nrp", [SEQ, D_MODEL], BF16, kind="ExternalInput").ap()
    qst = nc.dram_tensor("qst", [D_MODEL, D_MODEL], BF16, kind="ExternalInput").ap()
    kst = nc.dram_tensor("kst", [D_MODEL, D_MODEL], BF16, kind="ExternalInput").ap()
    vst = nc.dram_tensor("vst", [D_MODEL, VTOT], BF16, kind="ExternalInput").ap()
    ow = nc.dram_tensor("ow", [D_MODEL, D_MODEL], BF16, kind="ExternalInput").ap()
    # all biases host-packed into one row-replicated [128, 2080] tensor:
    # cols [0:8] qb by pair, [8:16] kb by pair, [16:1056] vb, [1056:2080] ob
    biases = nc.dram_tensor("biases", [P, 16 + VTOT + D_MODEL], F32, kind="ExternalInput").ap()
    out = nc.dram_tensor("out", [SEQ, D_MODEL], BF16, kind="ExternalOutput").ap()
    sums_dram = nc.dram_tensor("sums_scratch", [N_HEADS, SEQ], F32).ap()

    with tile.TileContext(nc) as tc:
        _kernel(tc, out, enc, nrp, qst, kst, vst, ow, biases,
                sums_dram=sums_dram)
    nc.compile()
    return nc


def _kernel(tc, out, enc, nrp, qst, kst, vst, ow, biases, sums_dram=None):
    nc = tc.nc

    smalls = tc.alloc_tile_pool(name="smalls", bufs=1)
    ident_bf = smalls.tile([P, P], BF16, tag="ident_bf", name="ident_bf")
    make_identity(nc, ident_bf)
    # sel[a][j, p] = 1 where j == (p // 64) * 8 + a: broadcasts the [16, 128]
    # reciprocal layout (row = (head, seg), col = q%128) to [128, q-seg a]
    sel = []
    self_f = smalls.tile([N_HEADS, P], F32, tag="self", name="self")
    for a in range(DCH):
        nc.gpsimd.memset(self_f, 0.0)
        nc.gpsimd.affine_select(
            out=self_f.rearrange("j (h c) -> j h c", h=2),
            in_=self_f.rearrange("j (h c) -> j h c", h=2),
            compare_op=mybir.AluOpType.not_equal,
            fill=1.0, base=-a,
            pattern=[[-8, 2], [0, D_HEAD]], channel_multiplier=1,
        )
        s_r = smalls.tile([N_HEADS, P], F32R, tag=f"sel{a}", name=f"sel{a}")
        nc.vector.tensor_copy(s_r, self_f)
        sel.append(s_r)
    bias_sb = smalls.tile([P, 16 + VTOT + D_MODEL], F32, tag="bias_sb", name="bias_sb")
    qb_col = bias_sb[:, 0:PAIRS]
    kb_col = bias_sb[:, PAIRS:2 * PAIRS]
    vb_bc = bias_sb[:, 16:16 + VTOT]
    ob_bc = bias_sb[:, 16 + VTOT:16 + VTOT + D_MODEL]

    # persistent weight tiles (bf16), prefetched on the scalar DMA queue in
    # consumption order (v projection first, output projection last)
    wpool = tc.alloc_tile_pool(name="weights", bufs=1, side="right")
    vw = [wpool.tile([P, VTOT], BF16, tag=f"vw{c}", name=f"vw{c}") for c in range(DCH)]
    qw = [wpool.tile([P, D_MODEL], BF16, tag=f"qw{c}", name=f"qw{c}") for c in range(DCH)]
    kw = [wpool.tile([P, D_MODEL], BF16, tag=f"kw{c}", name=f"kw{c}") for c in range(DCH)]
    owt = [wpool.tile([P, D_MODEL], BF16, tag=f"owt{c}", name=f"owt{c}") for c in range(DCH)]
    for c in range(DCH):
        nc.scalar.dma_start(out=vw[c], in_=vst[c * P:(c + 1) * P, :])
    for c in range(DCH):
        nc.scalar.dma_start(out=qw[c], in_=qst[c * P:(c + 1) * P, :])
    for c in range(DCH):
        nc.scalar.dma_start(out=kw[c], in_=kst[c * P:(c + 1) * P, :])
    for c in range(DCH):
        nc.scalar.dma_start(out=owt[c], in_=ow[c * P:(c + 1) * P, :])

    enc_t_pool = tc.alloc_tile_pool(name="encT", bufs=1, side="right")
    nrp_t_pool = tc.alloc_tile_pool(name="nrpT", bufs=1, side="right")
    encT = [enc_t_pool.tile([P, SEQ], BF16, tag=f"encT{c}", name=f"encT{c}") for c in range(DCH)]
    nrpT = [nrp_t_pool.tile([P, SEQ], BF16, tag=f"nrpT{c}", name=f"nrpT{c}") for c in range(DCH)]

    # ---- phase 1: dummy-matmul burst to lift the HAM clock gate, then
    #      transpose nrp into [d, s] (bf16). enc is transposed later,
    #      interleaved into the v projection so its DMA latency hides
    #      behind matmul work. Chunk-grouped 3/3/2 so the transpose PSUM
    #      tiles fit alongside the v-projection accumulator. ----
    with tc.tile_pool(name="warmps", bufs=1, space="PSUM") as wps:
        warm = wps.tile([P, P], F32, tag="warm", name="warm")
        for _ in range(32):
            nc.tensor.matmul(warm, ident_bf, ident_bf, start=True, stop=True,
                             skip_group_check=True)
    CG = ((0, 1, 2), (3, 4, 5), (6, 7))
    trin = tc.alloc_tile_pool(name="trin", bufs=8, side="right")
    pproj = tc.alloc_tile_pool(name="pproj", bufs=2, space="PSUM")
    trps = tc.alloc_tile_pool(name="trps", bufs=1, space="PSUM")

    def transpose_block(src, dst, tq):
        # two seq-tiles of src -> dst[c][:, tq*P:(tq+2)*P]
        s_ins = []
        for t in range(tq, tq + 2):
            s_in = trin.tile([P, D_MODEL], BF16, tag="s_in", name="s_in")
            nc.sync.dma_start(out=s_in, in_=src[t * P:(t + 1) * P, :])
            s_ins.append(s_in)
        for cg in CG:
            ptiles = {c: trps.tile([P, 2 * P], BF16, tag=f"tr{ci}", name=f"tr{ci}")
                      for ci, c in enumerate(cg)}
            for ti, s_in in enumerate(s_ins):
                for c in cg:
                    nc.tensor.transpose(
                        ptiles[c][:, ti * P:(ti + 1) * P],
                        s_in[:, c * P:(c + 1) * P],
                        ident_bf,
                    )
            for c in cg:
                nc.any.tensor_copy(dst[c][:, tq * P:(tq + 2) * P], ptiles[c])

    for tq in range(0, ST, 2):
        transpose_block(nrp, nrpT, tq)
    nc.sync.dma_start(out=bias_sb, in_=biases)

    qt_pool = tc.alloc_tile_pool(name="qt", bufs=1)
    kt_pool = tc.alloc_tile_pool(name="kt", bufs=1)
    qt = [qt_pool.tile([P, SEQ], BF16, tag=f"qt{g}", name=f"qt{g}") for g in range(PAIRS)]
    kt = [kt_pool.tile([P, SEQ], BF16, tag=f"kt{g}", name=f"kt{g}") for g in range(PAIRS)]
    va_pool = tc.alloc_tile_pool(name="va", bufs=1)
    va = [va_pool.tile([P, VTOT], BF16, tag=f"va{t}", name=f"va{t}") for t in range(ST)]
    zt_pool = tc.alloc_tile_pool(name="zt", bufs=1)
    zt = [zt_pool.tile([P, SEQ], BF16, tag=f"zt{k}", name=f"zt{k}") for k in range(DCH)]

    rpool = tc.alloc_tile_pool(name="rnorm", bufs=1)

    # ---- PE filler queue: (pair, early_ok, emit). Early slots sit right
    # behind fresh attention matmuls, so only latency-free work goes there.
    # Units are pair-tagged so everything pair g needs is force-drained
    # before its attention begins. ----
    filler = []
    proj_state = {}

    def proj_half_mms(g, w, bcol, dst, n0, chalf):
        key = (g, id(w), n0)
        if chalf == 0:
            proj_state[key] = pproj.tile([P, 512], F32, tag="pp", name="pp")
        pp = proj_state[key]
        for c in range(4 * chalf, 4 * chalf + 4):
            nc.tensor.matmul(
                pp,
                w[c][:, g * P:(g + 1) * P],
                encT[c][:, n0:n0 + 512],
                start=(c == 0), stop=(c == DCH - 1),
                skip_group_check=True,
            )
        if chalf == 1:
            del proj_state[key]
            nc.vector.tensor_scalar_add(
                out=dst[g][:, n0:n0 + 512], in0=pp, scalar1=bcol[:, g:g + 1],
            )

    def push_proj_pair(g):
        for w, bcol, dst in ((qw, qb_col, qt), (kw, kb_col, kt)):
            for n0 in (0, 512):
                for chalf in (0, 1):
                    filler.append((g, True,
                                   lambda g=g, w=w, bcol=bcol, dst=dst, n0=n0, chalf=chalf:
                                   proj_half_mms(g, w, bcol, dst, n0, chalf)))

    def push_norm_pair(pg):
        # reciprocal of softmax sums for pair pg: gather the two sum rows as
        # [16, 128] so the FD-bound reciprocal runs across partitions, then
        # broadcast to [128, SEQ] via the one-hot selector matmuls.
        s2 = rpool.tile([N_HEADS, P], F32, tag="s2", name="s2", bufs=2)
        nc.sync.dma_start(
            out=s2,
            in_=sums_dram[2 * pg:2 * pg + 2, :].rearrange("h (a c) -> (h a) c", c=P),
        )
        r2 = rpool.tile([N_HEADS, P], F32R, tag="r2", name="r2", bufs=2)
        with nc.allow_low_precision(reason="softmax denominators are O(1); fp32r rounding is fine"):
            nc.vector.reciprocal(out=r2, in_=s2)

        def apply(half, r2=r2, pg=pg):
            pb = pproj.tile([P, 512], F32, tag="pp", name="ppb")
            for a in range(4 * half, 4 * half + 4):
                nc.tensor.matmul(
                    pb[:, (a % 4) * P:(a % 4 + 1) * P], sel[a], r2,
                    start=True, stop=True, skip_group_check=True,
                )
            nc.vector.tensor_mul(
                zt[pg][:, half * 512:half * 512 + 512],
                zt[pg][:, half * 512:half * 512 + 512],
                pb,
            )

        for half in (0, 1):
            filler.append((None, False, lambda half=half: apply(half)))

    def pop_filler(allow_late):
        for idx, (pg, early_ok, emit) in enumerate(filler):
            if early_ok or allow_late:
                filler.pop(idx)
                emit()
                return

    def drain_pair(g):
        # everything pair g depends on must be emitted before its attention
        mine = [u for u in filler if u[0] == g]
        filler[:] = [u for u in filler if u[0] != g]
        for u in mine:
            u[2]()

    # ---- phase 2: v projection -> va [m, 16*65] with ones columns.
    # enc transpose blocks and the q/k projections for pairs 0 and 1 are
    # interleaved so the PE never idles on the enc DMA stream. The n0=0
    # projection halves only need enc seq-tiles 0-3, which are transposed
    # by t=3; n0=512 halves are force-drained at attention start. ----
    for g01 in (0, 1):
        for w, bcol, dst in ((qw, qb_col, qt), (kw, kb_col, kt)):
            for chalf in (0, 1):
                filler.append((g01, True,
                               lambda g=g01, w=w, bcol=bcol, dst=dst, chalf=chalf:
                               proj_half_mms(g, w, bcol, dst, 0, chalf)))
    for g01 in (0, 1):
        for w, bcol, dst in ((qw, qb_col, qt), (kw, kb_col, kt)):
            for chalf in (0, 1):
                filler.append((g01, True,
                               lambda g=g01, w=w, bcol=bcol, dst=dst, chalf=chalf:
                               proj_half_mms(g, w, bcol, dst, 512, chalf)))
    with tc.tile_pool(name="pv", bufs=1, space="PSUM") as pv:
        for t in range(ST):
            pt = pv.tile([P, VTOT], F32, tag="pv", name="pvt")
            for c in range(DCH):
                for n0 in range(0, VTOT, 512):
                    nw = min(512, VTOT - n0)
                    nc.tensor.matmul(
                        pt[:, n0:n0 + nw],
                        nrpT[c][:, t * P:(t + 1) * P],
                        vw[c][:, n0:n0 + nw],
                        start=(c == 0), stop=(c == DCH - 1),
                        skip_group_check=True,
                    )
            if t < 4:
                transpose_block(enc, encT, 2 * t)
            else:
                pop_filler(allow_late=True)
                pop_filler(allow_late=True)
            # vb_bc has the per-(h,dh) bias, with 1.0 in each ones-column slot;
            # matmul wrote 0 there (vst ones-columns are zero), so add gives 1.0
            nc.vector.tensor_add(va[t], pt, vb_bc)

    trps.release()
    trin.release()
    nrp_t_pool.release()

    # ---- phase 3: attention; later pairs' q/k projections and earlier
    #      pairs' softmax normalization ride the PE filler queue ----
    with tc.tile_pool(name="attn", bufs=3) as apool, \
         tc.tile_pool(name="ps_s", bufs=2, space="PSUM") as spool, \
         tc.tile_pool(name="ps_z", bufs=1, space="PSUM") as zpool:
        for h in range(N_HEADS):
            g, off = h // 2, (h % 2) * D_HEAD
            if h % 2 == 0:
                drain_pair(g)
                if g >= 1:
                    push_norm_pair(g - 1)
                if g + 2 < PAIRS:
                    push_proj_pair(g + 2)
            pz = zpool.tile([VW, SEQ], F32, tag="pz", name="pz")

            def av_mms(i, ae):
                q0 = i * P
                for n0, nw in _bank_splits(q0):
                    nc.tensor.matmul(
                        pz[:, n0:n0 + nw],
                        va[i][:, h * VW:(h + 1) * VW],
                        ae[:, n0:n0 + nw],
                        start=(i == 0), stop=(i == ST - 1),
                        skip_group_check=True,
                    )

            pend = None
            for i in range(ST):
                q0 = i * P
                ps = spool.tile([P, SEQ], F32, tag="ps", name="ps")
                ae = apool.tile([P, SEQ], BF16, tag="ae", name="ae")
                for n0, nw in _bank_splits(q0):
                    nc.tensor.matmul(
                        ps[:, n0:n0 + nw],
                        kt[g][off:off + D_HEAD, q0:q0 + P],
                        qt[g][off:off + D_HEAD, n0:n0 + nw],
                        start=True, stop=True,
                        skip_group_check=True,
                    )
                if i % 2 == 0:
                    pop_filler(allow_late=(i >= 4))
                nc.scalar.activation(
                    out=ae[:, q0:SEQ], in_=ps[:, q0:SEQ],
                    func=AF.Exp, scale=float(SCALE),
                )
                # causal diag mask: zero ae[m, q] where m > q (gpsimd, off
                # the PE critical path; exp of unmasked scores is harmless)
                nc.gpsimd.affine_select(
                    out=ae[:, q0:q0 + P], in_=ae[:, q0:q0 + P],
                    compare_op=mybir.AluOpType.is_ge,
                    fill=0.0, base=0,
                    pattern=[[1, P]], channel_multiplier=-1,
                )
                # attn@v delayed one chunk so exp latency hides behind PE work
                if pend is not None:
                    av_mms(*pend)
                pend = (i, ae)
            av_mms(*pend)
            # denominator row first (feeds the normalization chain), then
            # stash unnormalized zT; both free the PSUM slot
            srow = rpool.tile([1, SEQ], F32, tag="srow", name="srow", bufs=2)
            nc.vector.tensor_copy(srow, pz[D_HEAD:VW, :])
            nc.sync.dma_start(out=sums_dram[h:h + 1, :], in_=srow)
            nc.vector.tensor_copy(zt[g][off:off + D_HEAD, :], pz[0:D_HEAD, :])

        push_norm_pair(PAIRS - 1)

    # ---- phase 4: output projection out[s, d] = zt.T @ O + ob.
    # The first two seq-tiles' k=0..6 accumulations overlap the final
    # pair's normalization chain; their k=7 matmuls are issued after it.
    with tc.tile_pool(name="outsb", bufs=3) as outsb, \
         tc.tile_pool(name="po", bufs=2, space="PSUM") as po:
        def out_mms(pt, t, k):
            for n0 in range(0, D_MODEL, 512):
                nc.tensor.matmul(
                    pt[:, n0:n0 + 512],
                    zt[k][:, t * P:(t + 1) * P],
                    owt[k][:, n0:n0 + 512],
                    start=(k == 0), stop=(k == DCH - 1),
                    skip_group_check=True,
                )

        def out_drain(pt, t):
            # bf16 output (upcast host-side) halves the writeback traffic,
            # striped over three DMA queues so the tail drains fast
            ot = outsb.tile([P, D_MODEL], BF16, tag="ot", name="ot")
            nc.vector.tensor_add(ot, pt, ob_bc)
            eng = (nc.sync, nc.scalar, nc.gpsimd)[t % 3]
            eng.dma_start(out=out[t * P:(t + 1) * P, :], in_=ot)

        head_pts = []
        for t in (0, 1):
            pt = po.tile([P, D_MODEL], F32, tag="po", name="pot")
            head_pts.append(pt)
            for k in range(DCH - 1):
                out_mms(pt, t, k)
        while filler:
            pop_filler(allow_late=True)
        for t in (0, 1):
            out_mms(head_pts[t], t, DCH - 1)
            out_drain(head_pts[t], t)
        for t in range(2, ST):
            pt = po.tile([P, D_MODEL], F32, tag="po", name="pot")
            for k in range(DCH):
                out_mms(pt, t, k)
            out_drain(pt, t)

    pproj.release()
    for pool in (rpool, zt_pool, va_pool, kt_pool, qt_pool, enc_t_pool, wpool, smalls):
        pool.release()


def _get_program():
    if "nc" not in _CACHE:
        _CACHE["nc"] = _build_program()
    return _CACHE["nc"]


def _pack_weights(Qs, Qbs, Ks, Kbs, Vs, Vbs, O, Ob):
    f = np.float32
    qst = np.ascontiguousarray(np.transpose(np.asarray(Qs, f), (1, 0, 2)).reshape(D_MODEL, D_MODEL)).astype(NPBF16)
    kst = np.ascontiguousarray(np.transpose(np.asarray(Ks, f), (1, 0, 2)).reshape(D_MODEL, D_MODEL)).astype(NPBF16)
    vst = np.zeros((D_MODEL, VTOT), f)
    vb = np.zeros((VTOT,), f)
    Vs = np.asarray(Vs, f)
    Vbs = np.asarray(Vbs, f)
    for h in range(N_HEADS):
        vst[:, h * VW:h * VW + D_HEAD] = Vs[h]
        vb[h * VW:h * VW + D_HEAD] = Vbs[h]
        vb[h * VW + D_HEAD] = 1.0
    vst = vst.astype(NPBF16)
    ow = np.ascontiguousarray(np.asarray(O, f).reshape(D_MODEL, D_MODEL)).astype(NPBF16)
    qbf = np.asarray(Qbs, f).reshape(D_MODEL)
    kbf = np.asarray(Kbs, f).reshape(D_MODEL)
    obf = np.asarray(Ob, f).reshape(D_MODEL)
    # row-replicated bias pack: [0:8] qb by (pair, partition), [8:16] kb,
    # [16:1056] vb broadcast, [1056:2080] ob broadcast
    biases = np.empty((P, 16 + VTOT + D_MODEL), f)
    biases[:, 0:PAIRS] = qbf.reshape(PAIRS, P).T
    biases[:, PAIRS:2 * PAIRS] = kbf.reshape(PAIRS, P).T
    biases[:, 16:16 + VTOT] = vb[None, :]
    biases[:, 16 + VTOT:] = obf[None, :]
    return qst, kst, vst, ow, np.ascontiguousarray(biases)


def kernel(normalized_resid_pre, encoder_output, Qs, Qbs, Ks, Kbs, Vs, Vbs, O, Ob,
           _trace=False, _trace_kwargs=None):
    nc = _get_program()
    qst, kst, vst, ow, biases = _pack_weights(Qs, Qbs, Ks, Kbs, Vs, Vbs, O, Ob)
    enc = np.asarray(encoder_output, np.float32).astype(NPBF16)
    nrp = np.asarray(normalized_resid_pre, np.float32).astype(NPBF16)
    in_maps = []
    for b in range(BATCH):
        in_maps.append({
            "enc": np.ascontiguousarray(enc[b]),
            "nrp": np.ascontiguousarray(nrp[b]),
            "qst": qst, "kst": kst, "vst": vst, "ow": ow,
            "biases": biases,
        })
    res = run_bass_kernel_spmd(
        nc, in_maps, list(range(BATCH)),
        trace=_trace, **(_trace_kwargs or {}),
    )
    out = np.stack([np.asarray(res.results[b]["out"]).astype(np.float32) for b in range(BATCH)], axis=0)
    if _trace:
        _CACHE["last_results"] = res
    return out
